# revision 57
# baseline (speedup 1.0000x reference)
"""GTrXL layer (TransformerXL attention + GRU gating) on 8 TRN2 NeuronCores.

Sharding: pure data-parallel over batch (BS=8 -> 1 batch element per core).
No collectives. Per-core Bass/Tile kernel computes the full layer for its
batch element.

v2: fp8(e4m3) DoubleRow matmuls for all dense GEMMs (K=256 per instruction
at 0.5 cycles/row = 4x bf16 PE throughput), attention scores in bf16 with
the rel-shift DMA trick in fp8, XBAR dma-transpose for the softmax
transposition, an extended-V matmul that produces the softmax denominator
for free, and LN affine folding into the consuming weights (host-side).

Scales (host pre-scales; exact powers of two, no extra rounding):
  - fp8 weights stored x32
  - fp8 activations feeding GRU/MLP/proj matmuls stored x8
  - every PSUM readout rescales back to natural units.
Scores/K/V/Q/R stay natural-scale bf16; the rel-shift pad is -240 (fp8 min)
so exp((content-240)*0.125) == 0 covers the causal mask for free.
"""

import sys

if '/opt/trn_rl_repo' not in sys.path:
    sys.path.insert(0, '/opt/trn_rl_repo')

import numpy as np
import ml_dtypes

import concourse.bass as bass
import concourse.tile as tile
from concourse import bacc, mybir
from concourse.bass_utils import run_bass_kernel_spmd
from concourse.masks import make_identity

F8 = mybir.dt.float8e4
BF16 = mybir.dt.bfloat16
F32 = mybir.dt.float32
NF8 = ml_dtypes.float8_e4m3
NBF = ml_dtypes.bfloat16

HEAD_NUM, HEAD_DIM = 16, 64
D, HID = 1024, 4096
CUR, PREV, BS = 512, 512, 8
FULL = CUR + PREV
EPS = 1e-5
SCALE = 1.0 / (HEAD_DIM ** 0.5)
P = 128
DC = D // P          # 8 feature chunks
HC = HID // P        # 32 hidden chunks
TCF = FULL // P      # 8 full-token chunks
TCC = CUR // P       # 4 query-token chunks
NEGPAD = -240.0      # fp8 e4m3 most-negative finite
WS = 32.0            # weight scale
AS = 8.0             # activation scale
RS = 1.0 / (WS * AS)

AluOp = mybir.AluOpType
Act = mybir.ActivationFunctionType
DR = mybir.MatmulPerfMode.DoubleRow


def _dram_in(dram, name, shape, dtype):
    return dram.tile(list(shape), dtype, kind="ExternalInput", name=name,
                     uniquify=False)


def _build():
    nc = bacc.Bacc("TRN2", target_bir_lowering=False)
    with tile.TileContext(nc) as tc:
        _emit(nc, tc)
    nc.compile()
    return nc


def _wid(ic):
    """causal key width for query chunk ic (keys j <= i + PREV)"""
    return (ic + 5) * P


def _qlo(jc):
    """first valid query row for key chunk jc"""
    return max(0, (jc - 4) * P)


def _emit(nc, tc):
    from contextlib import ExitStack

    with ExitStack() as root:
        dram = root.enter_context(tc.tile_pool(name="io", bufs=1, space="DRAM"))

        # ---------------- DRAM I/O ----------------
        x_full = _dram_in(dram, "x_full", (FULL, D), BF16)
        inpT_d = _dram_in(dram, "inpT", (D, CUR), F32)
        posT_d = _dram_in(dram, "posT", (D, FULL), F8)

        wkv_d = _dram_in(dram, "Wkv", (D, 2 * D), F8)
        wq_d = _dram_in(dram, "Wq", (D, D), F8)
        wpos_d = _dram_in(dram, "Wpos", (D, D), F8)
        wproj_d = _dram_in(dram, "Wproj", (D, D), F8)
        gw_d = {}
        for g in (1, 2):
            for m in ("Wr", "Ur", "Wz", "Uz", "Wg", "Ug"):
                gw_d[(g, m)] = _dram_in(dram, f"g{g}_{m}", (D, D), F8)
        w1_d = _dram_in(dram, "mlp_W1", (D, HID), F8)
        w2_d = _dram_in(dram, "mlp_W2", (HID, D), F8)

        biases_d = _dram_in(dram, "biases_t", (P, 96), F32)
        bkvV_d = _dram_in(dram, "bkvV32_row", (1, D), BF16)

        out_d = dram.tile([CUR, D], F32, kind="ExternalOutput", name="out",
                          uniquify=False)

        n_scr = 16
        SCRB = P * 1536  # elements per scratch block
        scr_all = dram.tile([n_scr, P, 1536], F8, name="scr_all")
        scr = [scr_all[s] for s in range(n_scr)]

        # ---------------- constants ----------------
        const = root.enter_context(tc.tile_pool(name="const", bufs=1))
        ident_f = const.tile([P, P], F32)
        make_identity(nc, ident_f)
        ident_8 = const.tile([P, P], F8)
        make_identity(nc, ident_8)
        ident_b = const.tile([P, P], BF16)
        make_identity(nc, ident_b)
        ones_row = const.tile([1, P], BF16)
        nc.vector.memset(ones_row, 1.0)
        ones_red = const.tile([P, 1], BF16)
        nc.vector.memset(ones_red, 1.0)
        eps_t = const.tile([P, 1], F32)
        nc.vector.memset(eps_t, EPS)
        eps64_t = const.tile([1, 1], F32)
        nc.vector.memset(eps64_t, EPS / 64.0)

        def cload(name, dref, shape, dtype=F32):
            t = const.tile(list(shape), dtype, name=name)
            nc.sync.dma_start(out=t, in_=dref[:])
            return t

        biases_sb = cload("biases_sb", biases_d, (P, 96))
        bkvV_sb = cload("bkvV_sb", bkvV_d, (1, D), BF16)
        bkvK_sb = biases_sb[:, 0:8]
        su_sb = biases_sb[:, 8:16]
        sv_sb = biases_sb[:, 16:24]
        bpos_sb = biases_sb[:, 24:32]
        bproj8_sb = biases_sb[:, 32:40]
        b2_8_sb = biases_sb[:, 40:48]
        nbg1_sb = biases_sb[:, 48:56]
        nbg2_sb = biases_sb[:, 56:64]
        b1f64_sb = biases_sb[:, 64:96]

        padw4 = const.tile([P, 4, 512], F8)
        nc.vector.memset(padw4, NEGPAD)

        # psum pools: big (scores) 3x 2 banks, small 2x 1 bank = 8 banks
        psum_b = root.enter_context(tc.tile_pool(name="psum_b", bufs=3, space="PSUM"))
        psum_s = root.enter_context(tc.tile_pool(name="psum_s", bufs=2, space="PSUM"))

        def PB():
            return psum_b.tile([P, 1024], F32, name="pbig", tag="pbig")

        def PS():
            return psum_s.tile([P, 512], F32, name="ps", tag="ps")

        def mk(name, shape, dtype, side):
            t, fr = tc.tile(list(shape), dtype, name=name, side=side)
            return t, fr

        # ---- engine-cycled psum readout: (ps * scale) + bias -> out ----
        def ro(eng, out, ps, scale, bias_ap, relu=False):
            if relu:
                nc.scalar.activation(out=out, in_=ps, func=Act.Relu,
                                     scale=scale, bias=bias_ap)
            elif eng == 's':
                nc.scalar.activation(out=out, in_=ps, func=Act.Identity,
                                     scale=scale, bias=bias_ap)
            elif eng == 'v':
                nc.vector.tensor_scalar(out=out, in0=ps, scalar1=scale,
                                        scalar2=bias_ap, op0=AluOp.mult,
                                        op1=AluOp.add)
            else:
                nc.gpsimd.tensor_scalar(out=out, in0=ps, scalar1=scale,
                                        scalar2=bias_ap, op0=AluOp.mult,
                                        op1=AluOp.add)

        # ========== Phase 1: LN1 (no affine) -> x1T fp8; inpT loads =====
        inpT_f, fr_inpf = mk("inpT_f", (P, DC, CUR), F32, "right")
        inpT8, fr_inp8 = mk("inpT8", (P, DC, CUR), F8, "right")
        x1T, fr_x1T = mk("x1T", (P, DC, FULL), F8, "left")

        with ExitStack() as ph:
            xw = ph.enter_context(tc.tile_pool(name="xw", bufs=3, side="right"))
            st = ph.enter_context(tc.tile_pool(name="st", bufs=3, side="right"))
            x_t = x_full[:].rearrange("(tc p) d -> p tc d", p=P)
            for tcx in range(TCF):
                xt = xw.tile([P, D], BF16, name="xt")
                nc.sync.dma_start(out=xt, in_=x_t[:, tcx, :])
                stats = st.tile([P, 2, 6], F32, name="stats")
                nc.vector.bn_stats(out=stats[:, 0, :], in_=xt[:, 0:512])
                nc.vector.bn_stats(out=stats[:, 1, :], in_=xt[:, 512:1024])
                mv = st.tile([P, 2], F32, name="mv")
                nc.vector.bn_aggr(out=mv, in_=stats)
                sd = st.tile([P, 1], F32, name="sd")
                nc.scalar.activation(out=sd, in_=mv[:, 1:2], func=Act.Sqrt,
                                     bias=eps_t)
                rstd = st.tile([P, 1], F32, name="rstd")
                nc.vector.reciprocal(out=rstd, in_=sd)
                nmr = st.tile([P, 1], F32, name="nmr")
                nc.vector.scalar_tensor_tensor(out=nmr, in0=mv[:, 0:1],
                                               scalar=-1.0, in1=rstd,
                                               op0=AluOp.mult, op1=AluOp.mult)
                xnb = xw.tile([P, D], BF16, name="xnb")
                nc.scalar.activation(out=xnb, in_=xt, func=Act.Identity,
                                     scale=rstd, bias=nmr)
                for half in range(2):
                    ptb = psum_s.tile([P, 512], BF16, name="ptb", tag="ps")
                    for q in range(4):
                        dcx = half * 4 + q
                        nc.tensor.transpose(ptb[:, q * P:(q + 1) * P],
                                            xnb[:, dcx * P:(dcx + 1) * P],
                                            ident_b)
                    dst = x1T[:, half * 4:(half + 1) * 4, tcx * P:(tcx + 1) * P]
                    srcv = ptb[:, :].rearrange("p (a b) -> p a b", a=4)
                    if half == 0:
                        nc.vector.tensor_copy(dst, srcv)
                    else:
                        nc.scalar.copy(dst, srcv)

        # ========== Phase 2+3: QKVR GEMMs fused with attention ==========
        # Emission order: Q, R first (so attention "fronts" = pos-score
        # scratch roundtrips can start), then fronts 0..2, then K/V (fills
        # hide the first scratch latencies), then the back/front pipeline.
        kT, fr_kT = mk("kT", (P, DC, FULL), BF16, "right")
        v_ext, fr_v = mk("v_ext", (P, TCF, HEAD_NUM, 65), BF16, "right")
        rT, fr_rT = mk("rT", (P, DC, FULL), BF16, "right")
        quT, fr_quT = mk("quT", (P, DC, CUR), BF16, "right")
        qvT, fr_qvT = mk("qvT", (P, DC, CUR), BF16, "right")
        nc.vector.memset(v_ext[:, :, :, 64:65], 0.125)

        with ExitStack() as ph:
            wqp = ph.enter_context(tc.tile_pool(name="wqp", bufs=1, side="right"))
            wq = wqp.tile([P, DC, D], F8)
            wq_ap = wq_d[:].rearrange("(kc p) n -> p kc n", p=P)
            nc.sync.dma_start(out=wq, in_=wq_ap)
            for np2 in range(DC // 2):
                qp = PB()
                for sub in range(2):
                    n = np2 * 2 + sub
                    for t in range(4):
                        nc.tensor.matmul(
                            qp[:, sub * 512:(sub + 1) * 512],
                            lhsT=wq[:, 2 * t:2 * t + 2, n * P:(n + 1) * P],
                            rhs=x1T[:, 2 * t:2 * t + 2, CUR:FULL],
                            start=(t == 0), stop=(t == 3), perf_mode=DR,
                            skip_group_check=True)
                for sub in range(2):
                    n = np2 * 2 + sub
                    sl = qp[:, sub * 512:(sub + 1) * 512]
                    ro('v', quT[:, n, :], sl, 1.0 / WS, su_sb[:, n:n + 1])
                    ro('s', qvT[:, n, :], sl, 1.0 / WS, sv_sb[:, n:n + 1])
        with ExitStack() as ph:
            wpp = ph.enter_context(tc.tile_pool(name="wpp", bufs=1, side="right"))
            wpos = wpp.tile([P, DC, D], F8)
            wp_ap = wpos_d[:].rearrange("(kc p) n -> p kc n", p=P)
            nc.sync.dma_start(out=wpos, in_=wp_ap)
            posT_sb = wpp.tile([P, DC, FULL], F8)
            nc.sync.dma_start(out=posT_sb, in_=posT_d[:].rearrange("(kc p) f -> p kc f", p=P))
            for n in range(DC):
                rp = PB()
                for fh in range(2):
                    for t in range(4):
                        nc.tensor.matmul(
                            rp[:, fh * 512:(fh + 1) * 512],
                            lhsT=wpos[:, 2 * t:2 * t + 2, n * P:(n + 1) * P],
                            rhs=posT_sb[:, 2 * t:2 * t + 2,
                                        fh * 512:(fh + 1) * 512],
                            start=(t == 0), stop=(t == 3), perf_mode=DR,
                            skip_group_check=True)
                ro('v' if n % 2 else 's', rT[:, n, :], rp, 1.0 / WS,
                   bpos_sb[:, n:n + 1])

        # rel-shift pads for all 16 scratch blocks (before any shifted read)
        for g4 in range(4):
            pad_ap = bass.AP(tensor=scr_all.tensor,
                             offset=scr_all.offset + g4 * 4 * SCRB + 1024,
                             ap=[[1536, P], [SCRB, 4], [1, 512]])
            nc.sync.dma_start(out=pad_ap, in_=padw4)

        # inpT loads + x8 fp8 conversion (needed at GRU1; overlaps attention)
        nc.sync.dma_start(
            out=inpT_f, in_=inpT_d[:].rearrange("(kc p) t -> p kc t", p=P))
        for n in range(DC):
            if n % 2 == 0:
                nc.vector.tensor_scalar_mul(inpT8[:, n, :], inpT_f[:, n, :], AS)
            else:
                nc.gpsimd.tensor_scalar_mul(inpT8[:, n, :], inpT_f[:, n, :], AS)

        with ExitStack() as ph:
            shw = ph.enter_context(tc.tile_pool(name="shw", bufs=3, side="left"))
            pbw = ph.enter_context(tc.tile_pool(name="pbw", bufs=3, side="left"))
            rw = ph.enter_context(tc.tile_pool(name="rw", bufs=2, side="left"))
            wpr = ph.enter_context(tc.tile_pool(name="wpr", bufs=1, side="left"))
            gwp1 = ph.enter_context(tc.tile_pool(name="gw1", bufs=2, side="left"))

            def head_aps(h):
                ch, rb = h // 2, (h % 2) * HEAD_DIM
                return (quT[rb:rb + HEAD_DIM, ch, :],
                        qvT[rb:rb + HEAD_DIM, ch, :],
                        kT[rb:rb + HEAD_DIM, ch, :],
                        rT[rb:rb + HEAD_DIM, ch, :], ch, rb)

            def emit_front(h):
                """pos scores -> pb4 -> scratch write -> shifted read"""
                _, qvh, _, rh, ch, rb = head_aps(h)
                b0 = (h % 4) * 4  # scratch block base (4-head rotation)
                pb4 = pbw.tile([P, TCC, 1024], F8, name="pb4")
                for ic in range(TCC):
                    pp = PB()
                    for jh in range(2):
                        nc.tensor.matmul(pp[:, jh * 512:(jh + 1) * 512],
                                         lhsT=qvh[:, ic * P:(ic + 1) * P],
                                         rhs=rh[:, jh * 512:(jh + 1) * 512],
                                         start=True, stop=True,
                                         skip_group_check=True)
                    if ic < 3:
                        nc.vector.tensor_copy(pb4[:, ic, :], pp)
                    else:
                        nc.scalar.copy(pb4[:, ic, :], pp)
                sw_ap = bass.AP(tensor=scr_all.tensor,
                                offset=scr_all.offset + b0 * SCRB,
                                ap=[[1536, P], [SCRB, TCC], [1, 1024]])
                nc.sync.dma_start(out=sw_ap, in_=pb4)
                shp4 = shw.tile([P, TCC, 1024], F8, name="shp4")
                sr_ap = bass.AP(tensor=scr_all.tensor,
                                offset=scr_all.offset + b0 * SCRB + 511,
                                ap=[[1535, P], [SCRB, TCC], [1, 1024]])
                nc.sync.dma_start(out=shp4, in_=sr_ap)
                return shp4

            # fronts 0..2 start their scratch roundtrips before K/V
            pend = {}
            for h in range(3):
                pend[h] = emit_front(h)

            # ---- K and V (nested pool; fills hide front latencies) ----
            with ExitStack() as phkv:
                wkvp = phkv.enter_context(
                    tc.tile_pool(name="wkvp", bufs=1, side="right"))
                wkv = wkvp.tile([P, DC, 2 * D], F8)
                wr_ap = wkv_d[:].rearrange("(kc p) n -> p kc n", p=P)
                for hf in range(2):
                    nc.sync.dma_start(out=wkv[:, hf * 4:(hf + 1) * 4, :],
                                      in_=wr_ap[:, hf * 4:(hf + 1) * 4, :])
                for i in range(DC):
                    # K chunk n=i: out [128n, 1024t]; psum = 32 * k_nat
                    kp = PB()
                    for th in range(2):
                        for t in range(4):
                            nc.tensor.matmul(
                                kp[:, th * 512:(th + 1) * 512],
                                lhsT=wkv[:, 2 * t:2 * t + 2, i * P:(i + 1) * P],
                                rhs=x1T[:, 2 * t:2 * t + 2,
                                        th * 512:(th + 1) * 512],
                                start=(t == 0), stop=(t == 3), perf_mode=DR,
                                skip_group_check=True)
                    ro('v' if i % 2 else 's', kT[:, i, :], kp, 1.0 / WS,
                       bkvK_sb[:, i:i + 1])
                    # V chunk t=i: out [128t, 1024f] -> v_ext strided
                    vp = PB()
                    for nh in range(2):
                        for k in range(4):
                            nc.tensor.matmul(
                                vp[:, nh * 512:(nh + 1) * 512],
                                lhsT=x1T[:, 2 * k:2 * k + 2, i * P:(i + 1) * P],
                                rhs=wkv[:, 2 * k:2 * k + 2,
                                        D + nh * 512:D + (nh + 1) * 512],
                                start=(k == 0), stop=False, perf_mode=DR,
                                skip_group_check=True)
                        nc.tensor.matmul(vp[:, nh * 512:(nh + 1) * 512],
                                         lhsT=ones_row,
                                         rhs=bkvV_sb[:, nh * 512:(nh + 1) * 512],
                                         start=False, stop=True,
                                         skip_group_check=True)
                    nc.scalar.activation(
                        out=v_ext[:, i, :, 0:64],
                        in_=vp[:, :].rearrange("p (a b) -> p a b", a=16),
                        func=Act.Copy, scale=1.0 / WS)

            esw = ph.enter_context(tc.tile_pool(name="esw", bufs=2, side="left"))
            etw = ph.enter_context(tc.tile_pool(name="etw", bufs=2, side="left"))
            avT, fr_avT = mk("avT", (P, DC, CUR), F8, "left")

            def emit_back(h, shp4):
                """content + shift-add + exp + XBAR + AV + normalize"""
                quh, _, kh, _, ch, rb = head_aps(h)
                es = esw.tile([P, TCC, 1024], BF16, name="es")
                for ic in range(TCC):
                    w = _wid(ic)
                    cp = PB()
                    nc.tensor.matmul(cp[:, 0:512],
                                     lhsT=quh[:, ic * P:(ic + 1) * P],
                                     rhs=kh[:, 0:512], start=True, stop=False,
                                     skip_group_check=True)
                    nc.tensor.matmul(cp[:, 512:w],
                                     lhsT=quh[:, ic * P:(ic + 1) * P],
                                     rhs=kh[:, 512:w], start=True, stop=False,
                                     skip_group_check=True)
                    nc.tensor.matmul(cp[:, 0:512], lhsT=ident_8,
                                     rhs=shp4[:, ic, 0:512], start=False,
                                     stop=False, skip_group_check=True)
                    nc.tensor.matmul(cp[:, 512:w], lhsT=ident_8,
                                     rhs=shp4[:, ic, 512:w], start=False,
                                     stop=True, skip_group_check=True)
                    nc.scalar.activation(out=es[:, ic, 0:w], in_=cp[:, 0:w],
                                         func=Act.Exp, scale=SCALE)
                esT = etw.tile([P, TCC, TCF, P], BF16, name="esT")
                for ic in range(TCC):
                    w = _wid(ic)
                    nc.sync.dma_start_transpose(esT[:, ic, 0:w // P, :],
                                                es[:, ic, 0:w])
                av = psum_s.tile([P, 512], F32, name="av", tag="ps")
                for jc in range(TCF):
                    ic0 = _qlo(jc) // P
                    nc.tensor.matmul(av[0:65, ic0 * P:512],
                                     lhsT=v_ext[:, jc, h, :],
                                     rhs=esT[:, ic0:TCC, jc, :],
                                     start=(jc == 0), stop=(jc == TCF - 1),
                                     skip_group_check=True)
                recip = rw.tile([1, 512], F32, name="recip")
                nc.vector.reciprocal(out=recip, in_=av[64:65, :])
                recipB = rw.tile([HEAD_DIM, 512], F32, name="recipB")
                nc.gpsimd.partition_broadcast(recipB, recip)
                nc.vector.tensor_mul(avT[rb:rb + HEAD_DIM, ch, :],
                                     av[0:HEAD_DIM, :], recipB)

            # back/front software pipeline with woven weight prefetch
            wproj = wpr.tile([P, DC, D], F8)
            g1pre = {}
            for h in range(HEAD_NUM):
                emit_back(h, pend.pop(h))
                if h + 3 < HEAD_NUM:
                    pend[h + 3] = emit_front(h + 3)
                if h == 6:
                    nc.sync.dma_start(
                        out=wproj,
                        in_=wproj_d[:].rearrange("(kc p) n -> p kc n", p=P))
                if h == 10:
                    for m in ("Wr", "Ur"):
                        w = gwp1.tile([P, DC, D], F8, name=f"g1_{m}", tag="gwt")
                        nc.sync.dma_start(
                            out=w,
                            in_=gw_d[(1, m)][:].rearrange("(kc p) n -> p kc n", p=P))
                        g1pre[m] = w
            fr_qvT(); fr_quT(); fr_rT(); fr_v(); fr_kT()

            o1T_f, fr_o1f = mk("o1T_f", (P, DC, CUR), F32, "right")
            o1T_b, fr_o1b = mk("o1T_b", (P, DC, CUR), BF16, "right")
            o1T8, fr_o18 = mk("o1T8", (P, DC, CUR), F8, "right")
            a1T, fr_a1T = mk("a1T", (P, DC, CUR), F8, "right")
            for np2 in range(DC // 2):
                pp = PB()
                for sub in range(2):
                    n = np2 * 2 + sub
                    sl = pp[:, sub * 512:(sub + 1) * 512]
                    for t in range(4):
                        nc.tensor.matmul(
                            sl, lhsT=wproj[:, 2 * t:2 * t + 2, n * P:(n + 1) * P],
                            rhs=avT[:, 2 * t:2 * t + 2, :],
                            start=(t == 0), stop=(t == 3), perf_mode=DR,
                            skip_group_check=True)
                    # psum = 256*(av@Wproj); a1T = 8*relu(av@Wproj + bproj)
                    ro('s', a1T[:, n, :], sl, 1.0 / WS, bproj8_sb[:, n:n + 1],
                       relu=True)
            fr_avT()

            # LN2 sums accumulate inside GRU1's output loop (s1 = sum o1,
            # s2 = sum o1^2, both [1,512] chains in one psum tile's banks)
            s1t = psum_s.tile([P, 512], F32, name="s1t", tag="ps")
            s2t = psum_s.tile([P, 512], F32, name="s2t", tag="ps")
            s1 = s1t[0:1, :]
            s2 = s2t[0:1, :]
            sqw = ph.enter_context(tc.tile_pool(name="sqw", bufs=2, side="left"))

            def post1(n):
                sq = sqw.tile([P, 512], BF16, name="sq")
                nc.vector.tensor_mul(sq, o1T_b[:, n, :], o1T_b[:, n, :])
                nc.tensor.matmul(s1, lhsT=ones_red, rhs=o1T_b[:, n, :],
                                 start=(n == 0), stop=(n == DC - 1),
                                 skip_group_check=True)
                nc.tensor.matmul(s2, lhsT=ones_red, rhs=sq,
                                 start=(n == 0), stop=(n == DC - 1),
                                 skip_group_check=True)

            _gru(nc, tc, ph, PB, gw_d, 1, a1T, inpT8, inpT_f, nbg1_sb,
                 o1T_f, o1T_b, o1T8, post_n=post1, gwp=gwp1, pre=g1pre)
        fr_a1T(); fr_x1T()

        # ========== Phase 5: LN2 (no affine) -> x2T8 (x8 fp8) ==========
        x2T8, fr_x2T = mk("x2T8", (P, DC, CUR), F8, "right")
        with ExitStack() as ph:
            lw = ph.enter_context(tc.tile_pool(name="lw", bufs=2, side="left"))
            mean = lw.tile([1, 512], F32, name="mean")
            nc.vector.tensor_scalar_mul(mean, s1, 1.0 / D)
            m2m = lw.tile([1, 512], F32, name="m2m")
            nc.vector.tensor_scalar_mul(m2m, s2, 1.0 / D)
            var = lw.tile([1, 512], F32, name="var")
            nc.vector.scalar_tensor_tensor(out=var, in0=mean, scalar=1.0,
                                           in1=mean, op0=AluOp.mult,
                                           op1=AluOp.mult)
            nc.vector.tensor_sub(var, m2m, var)
            # sd8 = sqrt((var+eps)/64) = sd/8 ; recip -> 8/sd
            sd8 = lw.tile([1, 512], F32, name="sd8")
            nc.scalar.activation(out=sd8, in_=var, func=Act.Sqrt,
                                 scale=1.0 / 64.0, bias=eps64_t)
            rstd8 = lw.tile([1, 512], F32, name="rstd8")
            nc.vector.reciprocal(out=rstd8, in_=sd8)
            meanB = lw.tile([P, 512], F32, name="meanB")
            nc.gpsimd.partition_broadcast(meanB, mean)
            rstd8B = lw.tile([P, 512], F32, name="rstd8B")
            nc.gpsimd.partition_broadcast(rstd8B, rstd8)
            meanB2 = bass.AP(tensor=meanB.tensor, offset=meanB.offset,
                             ap=[meanB[:].ap[0], [0, 2], [1, 512]])
            rstd8B2 = bass.AP(tensor=rstd8B.tensor, offset=rstd8B.offset,
                              ap=[rstd8B[:].ap[0], [0, 2], [1, 512]])
            for np2 in range(DC // 2):
                t1 = lw.tile([P, 2, 512], F32, name="t1")
                sl_i = o1T_f[:, 2 * np2:2 * np2 + 2, :]
                sl_o = x2T8[:, 2 * np2:2 * np2 + 2, :]
                if np2 % 2 == 0:
                    nc.vector.tensor_sub(t1, sl_i, meanB2)
                    nc.vector.tensor_mul(sl_o, t1, rstd8B2)
                else:
                    nc.gpsimd.tensor_sub(t1, sl_i, meanB2)
                    nc.gpsimd.tensor_mul(sl_o, t1, rstd8B2)

        # ========== Phase 6: MLP ========================================
        m2T, fr_m2T = mk("m2T", (P, DC, CUR), F8, "left")
        with ExitStack() as ph6:
            w1p = ph6.enter_context(tc.tile_pool(name="w1p", bufs=1, side="right"))
            m1w = ph6.enter_context(tc.tile_pool(name="m1w", bufs=1, side="right"))
            w2p = ph6.enter_context(tc.tile_pool(name="w2p", bufs=1, side="right"))
            w1 = w1p.tile([P, DC, HID], F8)
            w1_ap = w1_d[:].rearrange("(kc p) n -> p kc n", p=P)
            for hf in range(2):
                nc.sync.dma_start(out=w1[:, hf * 4:(hf + 1) * 4, :],
                                  in_=w1_ap[:, hf * 4:(hf + 1) * 4, :])
            w2 = w2p.tile([P, HC, D], F8)
            w2_ap = w2_d[:].rearrange("(kc p) n -> p kc n", p=P)
            for hf in range(2):
                nc.sync.dma_start(out=w2[:, hf * 16:(hf + 1) * 16, :],
                                  in_=w2_ap[:, hf * 16:(hf + 1) * 16, :])
            m1T = m1w.tile([P, HC, 512], F8)
            for np2 in range(HC // 2):
                pp = PB()
                for sub in range(2):
                    n = np2 * 2 + sub
                    sl = pp[:, sub * 512:(sub + 1) * 512]
                    for t in range(4):
                        nc.tensor.matmul(
                            sl, lhsT=w1[:, 2 * t:2 * t + 2, n * P:(n + 1) * P],
                            rhs=x2T8[:, 2 * t:2 * t + 2, :],
                            start=(t == 0), stop=(t == 3), perf_mode=DR,
                            skip_group_check=True)
                    # W1 stored x8, x2 x8 -> psum = 64*(x2@W1f);
                    # m1T = 64*relu(x2@W1f + b1f) via one (add,max) op
                    nc.vector.tensor_scalar(out=m1T[:, n, :], in0=sl,
                                            scalar1=b1f64_sb[:, n:n + 1],
                                            scalar2=0.0, op0=AluOp.add,
                                            op1=AluOp.max)
            for np2 in range(DC // 2):
                pp = PB()
                for sub in range(2):
                    n = np2 * 2 + sub
                    sl = pp[:, sub * 512:(sub + 1) * 512]
                    for t in range(16):
                        nc.tensor.matmul(
                            sl, lhsT=w2[:, 2 * t:2 * t + 2, n * P:(n + 1) * P],
                            rhs=m1T[:, 2 * t:2 * t + 2, :],
                            start=(t == 0), stop=(t == 15), perf_mode=DR,
                            skip_group_check=True)
                    # psum = 64*32*(m1@W2) = 2048x ; m2T = 8*relu(...)
                    ro('s', m2T[:, n, :], sl, 1.0 / 256.0, b2_8_sb[:, n:n + 1],
                       relu=True)
        fr_x2T()

        # ========== Phase 7: GRU2 =======================================
        o2T_f, fr_o2 = mk("o2T_f", (P, DC, CUR), F32, "left")
        with ExitStack() as ph:
            ow = ph.enter_context(tc.tile_pool(name="ow", bufs=2, side="left"))

            def post2(n):
                # transpose this output feature chunk and stream it out
                pt = PS()
                for q in range(TCC):
                    nc.tensor.transpose(pt[:, q * P:(q + 1) * P],
                                        o2T_f[:, n, q * P:(q + 1) * P],
                                        ident_f)
                on = ow.tile([P, TCC, P], F32, name="on")
                if n % 2 == 0:
                    nc.vector.tensor_copy(on, pt[:, :].rearrange(
                        "p (a b) -> p a b", a=TCC))
                else:
                    nc.scalar.copy(on, pt[:, :].rearrange(
                        "p (a b) -> p a b", a=TCC))
                o_ap = bass.AP(tensor=out_d.tensor,
                               offset=out_d.offset + n * P,
                               ap=[[D, P], [P * D, TCC], [1, P]])
                nc.sync.dma_start(out=o_ap, in_=on)

            _gru(nc, tc, ph, PB, gw_d, 2, m2T, o1T8, o1T_f, nbg2_sb,
                 o2T_f, None, None, post_n=post2)
        fr_o18(); fr_o1b(); fr_o1f(); fr_inp8(); fr_inpf()

        fr_o2(); fr_m2T()


def _gru(nc, tc, ph, PB, gw_d, g, yT8, xT8, xT_f, nbg_sb, oT_f, oT_b, oT8,
         post_n=None, gwp=None, pre=None):
    """GRU gate: yT8/xT8 fp8 at 8x scale; psum = 256*(y@W + x@U)."""
    if gwp is None:
        gwp = ph.enter_context(tc.tile_pool(name=f"gw{g}", bufs=3, side="left"))
    gtmp = ph.enter_context(tc.tile_pool(name=f"gt{g}", bufs=2, side="left"))
    gper = ph.enter_context(tc.tile_pool(name=f"gp{g}", bufs=1, side="left"))
    pre = pre or {}

    def loadw(m):
        if m in pre:
            return pre[m]
        w = gwp.tile([P, DC, D], F8, name=f"gwt_{m}", tag="gwt")
        ap = gw_d[(g, m)][:].rearrange("(kc p) n -> p kc n", p=P)
        nc.sync.dma_start(out=w, in_=ap)
        return w

    def gate_psum(ps, w, u, n, rhs_x):
        for t in range(4):
            nc.tensor.matmul(ps, lhsT=w[:, 2 * t:2 * t + 2, n * P:(n + 1) * P],
                             rhs=yT8[:, 2 * t:2 * t + 2, :],
                             start=(t == 0), stop=False, perf_mode=DR,
                             skip_group_check=True)
        for t in range(4):
            nc.tensor.matmul(ps, lhsT=u[:, 2 * t:2 * t + 2, n * P:(n + 1) * P],
                             rhs=rhs_x[:, 2 * t:2 * t + 2, :],
                             start=False, stop=(t == 3), perf_mode=DR,
                             skip_group_check=True)

    wr, ur = loadw("Wr"), loadw("Ur")
    rx8 = gper.tile([P, DC, 512], F8, name="rx8")
    for np2 in range(DC // 2):
        pp = PB()
        for sub in range(2):
            n = np2 * 2 + sub
            sl = pp[:, sub * 512:(sub + 1) * 512]
            gate_psum(sl, wr, ur, n, xT8)
            rr = gtmp.tile([P, 512], F32, name="rr")
            nc.scalar.activation(out=rr, in_=sl, func=Act.Sigmoid, scale=RS)
            # rx8 = (8*r) * x
            nc.vector.scalar_tensor_tensor(out=rx8[:, n, :], in0=rr, scalar=AS,
                                           in1=xT_f[:, n, :], op0=AluOp.mult,
                                           op1=AluOp.mult)
    wz, uz = loadw("Wz"), loadw("Uz")
    zt = gper.tile([P, DC, 512], BF16, name="zt")
    for np2 in range(DC // 2):
        pp = PB()
        for sub in range(2):
            n = np2 * 2 + sub
            sl = pp[:, sub * 512:(sub + 1) * 512]
            gate_psum(sl, wz, uz, n, xT8)
            nc.scalar.activation(out=zt[:, n, :], in_=sl, func=Act.Sigmoid,
                                 scale=RS, bias=nbg_sb[:, n:n + 1])
    wg, ug = loadw("Wg"), loadw("Ug")
    for np2 in range(DC // 2):
        pp = PB()
        for sub in range(2):
            n = np2 * 2 + sub
            sl = pp[:, sub * 512:(sub + 1) * 512]
            gate_psum(sl, wg, ug, n, rx8)
            ht = gtmp.tile([P, 512], BF16, name="ht")
            nc.scalar.activation(out=ht, in_=sl, func=Act.Tanh, scale=RS)
            d1 = gtmp.tile([P, 512], BF16, name="d1")
            nc.gpsimd.tensor_sub(d1, ht, xT_f[:, n, :])
            zd = gtmp.tile([P, 512], BF16, name="zd")
            nc.vector.tensor_mul(zd, d1, zt[:, n, :])
            nc.vector.tensor_add(oT_f[:, n, :], zd, xT_f[:, n, :])
            if oT_b is not None:
                nc.gpsimd.tensor_copy(oT_b[:, n, :], oT_f[:, n, :])
            if oT8 is not None:
                nc.gpsimd.tensor_scalar_mul(oT8[:, n, :], oT_f[:, n, :], AS)
            if post_n is not None:
                post_n(n)


_NC_CACHE = {}


def _get_nc():
    if "nc" not in _NC_CACHE:
        _NC_CACHE["nc"] = _build()
    return _NC_CACHE["nc"]


def _chunk_t(vec):
    n = vec.shape[0] // P
    return np.ascontiguousarray(vec.reshape(n, P).T.astype(np.float32))


def _f8(x, s=WS):
    return np.asarray(np.asarray(x, np.float32) * s, NF8)


def _prep(inputs):
    f32 = np.float32
    inp = np.asarray(inputs["inputs"], f32)
    mem = np.asarray(inputs["memory"], f32)
    pos = np.asarray(inputs["pos_embedding"], f32)[:, 0, :]

    g1 = np.asarray(inputs["ln1_g"], f32)
    b1 = np.asarray(inputs["ln1_b"], f32)
    g2 = np.asarray(inputs["ln2_g"], f32)
    b2 = np.asarray(inputs["ln2_b"], f32)
    Wkv = np.asarray(inputs["Wkv"], f32)
    Wq = np.asarray(inputs["Wq"], f32)
    W1 = np.asarray(inputs["mlp_W1"], f32)

    Wkv_f = g1[:, None] * Wkv
    bkv_f = b1 @ Wkv + np.asarray(inputs["bkv"], f32)
    Wq_f = g1[:, None] * Wq
    bq_f = b1 @ Wq + np.asarray(inputs["bq"], f32)
    W1_f = g2[:, None] * W1
    b1_f = b2 @ W1 + np.asarray(inputs["mlp_b1"], f32)

    u_flat = np.asarray(inputs["u"], f32).reshape(-1)
    v_flat = np.asarray(inputs["v"], f32).reshape(-1)

    shared = {
        "posT": np.ascontiguousarray(pos.T).astype(NF8),
        "Wkv": _f8(Wkv_f), "Wq": _f8(Wq_f), "Wpos": _f8(inputs["Wpos"]),
        "Wproj": _f8(inputs["Wproj"]),
        "mlp_W1": _f8(W1_f, 8.0), "mlp_W2": _f8(inputs["mlp_W2"]),
        "bkvV32_row": (bkv_f[D:2 * D] * WS).reshape(1, D).astype(NBF),
        "biases_t": np.concatenate([
            _chunk_t(bkv_f[0:D]),
            _chunk_t(bq_f + u_flat),
            _chunk_t(bq_f + v_flat),
            _chunk_t(np.asarray(inputs["bpos"], f32)),
            _chunk_t(np.asarray(inputs["bproj"], f32) * AS),
            _chunk_t(np.asarray(inputs["mlp_b2"], f32) * AS),
            _chunk_t(-np.asarray(inputs["g1_bg"], f32)),
            _chunk_t(-np.asarray(inputs["g2_bg"], f32)),
            _chunk_t(b1_f * 64.0),
        ], axis=1),
    }
    for g in (1, 2):
        for m in ("Wr", "Ur", "Wz", "Uz", "Wg", "Ug"):
            shared[f"g{g}_{m}"] = _f8(inputs[f"g{g}_{m}"])

    in_maps = []
    for b in range(BS):
        im = dict(shared)
        im["x_full"] = np.ascontiguousarray(
            np.concatenate([mem[:, b, :], inp[:, b, :]], axis=0)).astype(NBF)
        im["inpT"] = np.ascontiguousarray(inp[:, b, :].T)
        in_maps.append(im)
    return in_maps


def kernel(**inputs):
    nc = _get_nc()
    in_maps = _prep(inputs)
    res = run_bass_kernel_spmd(nc, in_maps, core_ids=list(range(BS)))
    out = np.stack([res.results[b]["out"] for b in range(BS)], axis=1)
    return np.ascontiguousarray(out.astype(np.float32))


if __name__ == "__main__":
    _get_nc()
    print("build+compile OK")


# revision 60
# speedup vs baseline: 1.0708x; 1.0708x over previous
"""GTrXL layer (TransformerXL attention + GRU gating) on 8 TRN2 NeuronCores.

Sharding: pure data-parallel over batch (BS=8 -> 1 batch element per core).
No collectives. Per-core Bass/Tile kernel computes the full layer for its
batch element.

v2: fp8(e4m3) DoubleRow matmuls for all dense GEMMs (K=256 per instruction
at 0.5 cycles/row = 4x bf16 PE throughput), attention scores in bf16 with
the rel-shift DMA trick in fp8, XBAR dma-transpose for the softmax
transposition, an extended-V matmul that produces the softmax denominator
for free, and LN affine folding into the consuming weights (host-side).

Scales (host pre-scales; exact powers of two, no extra rounding):
  - fp8 weights stored x32
  - fp8 activations feeding GRU/MLP/proj matmuls stored x8
  - every PSUM readout rescales back to natural units.
Scores/K/V/Q/R stay natural-scale bf16; the rel-shift pad is -240 (fp8 min)
so exp((content-240)*0.125) == 0 covers the causal mask for free.
"""

import sys

if '/opt/trn_rl_repo' not in sys.path:
    sys.path.insert(0, '/opt/trn_rl_repo')

import numpy as np
import ml_dtypes

import concourse.bass as bass
import concourse.tile as tile
from concourse import bacc, mybir
from concourse.bass_utils import run_bass_kernel_spmd
from concourse.masks import make_identity

F8 = mybir.dt.float8e4
BF16 = mybir.dt.bfloat16
F32 = mybir.dt.float32
NF8 = ml_dtypes.float8_e4m3
NBF = ml_dtypes.bfloat16

HEAD_NUM, HEAD_DIM = 16, 64
D, HID = 1024, 4096
CUR, PREV, BS = 512, 512, 8
FULL = CUR + PREV
EPS = 1e-5
SCALE = 1.0 / (HEAD_DIM ** 0.5)
P = 128
DC = D // P          # 8 feature chunks
HC = HID // P        # 32 hidden chunks
TCF = FULL // P      # 8 full-token chunks
TCC = CUR // P       # 4 query-token chunks
NEGPAD = -240.0      # fp8 e4m3 most-negative finite
WS = 32.0            # weight scale
AS = 8.0             # activation scale
RS = 1.0 / (WS * AS)

AluOp = mybir.AluOpType
Act = mybir.ActivationFunctionType
DR = mybir.MatmulPerfMode.DoubleRow


def _dram_in(dram, name, shape, dtype):
    return dram.tile(list(shape), dtype, kind="ExternalInput", name=name,
                     uniquify=False)


def _build():
    nc = bacc.Bacc("TRN2", target_bir_lowering=False)
    with tile.TileContext(nc) as tc:
        _emit(nc, tc)
    nc.compile()
    return nc


def _wid(ic):
    """causal key width for query chunk ic (keys j <= i + PREV)"""
    return (ic + 5) * P


def _qlo(jc):
    """first valid query row for key chunk jc"""
    return max(0, (jc - 4) * P)


def _emit(nc, tc):
    from contextlib import ExitStack

    with ExitStack() as root:
        dram = root.enter_context(tc.tile_pool(name="io", bufs=1, space="DRAM"))

        # ---------------- DRAM I/O ----------------
        x_full = _dram_in(dram, "x_full", (FULL, D), BF16)
        inpT_d = _dram_in(dram, "inpT", (D, CUR), F32)
        posT_d = _dram_in(dram, "posT", (D, FULL), F8)

        wkv_d = _dram_in(dram, "Wkv", (D, 2 * D), F8)
        wq_d = _dram_in(dram, "Wq", (D, D), F8)
        wpos_d = _dram_in(dram, "Wpos", (D, D), F8)
        wproj_d = _dram_in(dram, "Wproj", (D, D), F8)
        gw_d = {}
        for g in (1, 2):
            for m in ("Wr", "Ur", "Wz", "Uz", "Wg", "Ug"):
                gw_d[(g, m)] = _dram_in(dram, f"g{g}_{m}", (D, D), F8)
        w1_d = _dram_in(dram, "mlp_W1", (D, HID), F8)
        w2_d = _dram_in(dram, "mlp_W2", (HID, D), F8)

        biases_d = _dram_in(dram, "biases_t", (P, 96), F32)
        bkvV_d = _dram_in(dram, "bkvV32_row", (1, D), BF16)

        out_d = dram.tile([CUR, D], F32, kind="ExternalOutput", name="out",
                          uniquify=False)

        n_scr = 16
        SCRB = P * 1536  # elements per scratch block
        scr_all = dram.tile([n_scr, P, 1536], F8, name="scr_all")
        scr = [scr_all[s] for s in range(n_scr)]

        # ---------------- constants ----------------
        const = root.enter_context(tc.tile_pool(name="const", bufs=1))
        ident_f = const.tile([P, P], F32)
        make_identity(nc, ident_f)
        ident_8 = const.tile([P, P], F8)
        make_identity(nc, ident_8)
        ident_b = const.tile([P, P], BF16)
        make_identity(nc, ident_b)
        ones_row = const.tile([1, P], BF16)
        nc.vector.memset(ones_row, 1.0)
        ones_red = const.tile([P, 1], BF16)
        nc.vector.memset(ones_red, 1.0)
        eps_t = const.tile([P, 1], F32)
        nc.vector.memset(eps_t, EPS)
        eps64_t = const.tile([1, 1], F32)
        nc.vector.memset(eps64_t, EPS / 64.0)

        def cload(name, dref, shape, dtype=F32):
            t = const.tile(list(shape), dtype, name=name)
            nc.sync.dma_start(out=t, in_=dref[:])
            return t

        biases_sb = cload("biases_sb", biases_d, (P, 96))
        bkvV_sb = cload("bkvV_sb", bkvV_d, (1, D), BF16)
        bkvK_sb = biases_sb[:, 0:8]
        su_sb = biases_sb[:, 8:16]
        sv_sb = biases_sb[:, 16:24]
        bpos_sb = biases_sb[:, 24:32]
        bproj8_sb = biases_sb[:, 32:40]
        b2_8_sb = biases_sb[:, 40:48]
        nbg1_sb = biases_sb[:, 48:56]
        nbg2_sb = biases_sb[:, 56:64]
        b1f64_sb = biases_sb[:, 64:96]

        padw4 = const.tile([P, 4, 512], F8)
        nc.vector.memset(padw4, NEGPAD)

        # psum pools: big (scores) 3x 2 banks, small 2x 1 bank = 8 banks
        psum_b = root.enter_context(tc.tile_pool(name="psum_b", bufs=3, space="PSUM"))
        psum_s = root.enter_context(tc.tile_pool(name="psum_s", bufs=2, space="PSUM"))

        def PB():
            return psum_b.tile([P, 1024], F32, name="pbig", tag="pbig")

        def PS():
            return psum_s.tile([P, 512], F32, name="ps", tag="ps")

        def mk(name, shape, dtype, side):
            t, fr = tc.tile(list(shape), dtype, name=name, side=side)
            return t, fr

        # ---- engine-cycled psum readout: (ps * scale) + bias -> out ----
        def ro(eng, out, ps, scale, bias_ap, relu=False):
            if relu:
                nc.scalar.activation(out=out, in_=ps, func=Act.Relu,
                                     scale=scale, bias=bias_ap)
            elif eng == 's':
                nc.scalar.activation(out=out, in_=ps, func=Act.Identity,
                                     scale=scale, bias=bias_ap)
            elif eng == 'v':
                nc.vector.tensor_scalar(out=out, in0=ps, scalar1=scale,
                                        scalar2=bias_ap, op0=AluOp.mult,
                                        op1=AluOp.add)
            else:
                nc.gpsimd.tensor_scalar(out=out, in0=ps, scalar1=scale,
                                        scalar2=bias_ap, op0=AluOp.mult,
                                        op1=AluOp.add)

        # ========== Phase 1+2a: R first (LN1-independent), LN1 split ====
        # R = Wpos @ posT needs no LN1 output, so the PE computes it while
        # the vector/scalar engines run LN1. Query-token chunks (4..7) are
        # normalized first so the Q GEMM starts early too.
        inpT_f, fr_inpf = mk("inpT_f", (P, DC, CUR), F32, "right")
        inpT8, fr_inp8 = mk("inpT8", (P, DC, CUR), F8, "right")
        x1T, fr_x1T = mk("x1T", (P, DC, FULL), F8, "left")
        kT, fr_kT = mk("kT", (P, DC, FULL), BF16, "right")
        v_ext, fr_v = mk("v_ext", (P, TCF, HEAD_NUM, 65), BF16, "right")
        rT, fr_rT = mk("rT", (P, DC, FULL), BF16, "right")
        quT, fr_quT = mk("quT", (P, DC, CUR), BF16, "right")
        qvT, fr_qvT = mk("qvT", (P, DC, CUR), BF16, "right")
        nc.vector.memset(v_ext[:, :, :, 64:65], 0.125)

        with ExitStack() as ph:
            xw = ph.enter_context(tc.tile_pool(name="xw", bufs=4, side="right"))
            st = ph.enter_context(tc.tile_pool(name="st", bufs=3, side="right"))
            x_t = x_full[:].rearrange("(tc p) d -> p tc d", p=P)

            def ln1_chunk(tcx, xt):
                stats = st.tile([P, 2, 6], F32, name="stats")
                nc.vector.bn_stats(out=stats[:, 0, :], in_=xt[:, 0:512])
                nc.vector.bn_stats(out=stats[:, 1, :], in_=xt[:, 512:1024])
                mv = st.tile([P, 2], F32, name="mv")
                nc.vector.bn_aggr(out=mv, in_=stats)
                sd = st.tile([P, 1], F32, name="sd")
                nc.scalar.activation(out=sd, in_=mv[:, 1:2], func=Act.Sqrt,
                                     bias=eps_t)
                rstd = st.tile([P, 1], F32, name="rstd")
                nc.vector.reciprocal(out=rstd, in_=sd)
                nmr = st.tile([P, 1], F32, name="nmr")
                nc.vector.scalar_tensor_tensor(out=nmr, in0=mv[:, 0:1],
                                               scalar=-1.0, in1=rstd,
                                               op0=AluOp.mult, op1=AluOp.mult)
                xnb = xw.tile([P, D], BF16, name="xnb")
                nc.scalar.activation(out=xnb, in_=xt, func=Act.Identity,
                                     scale=rstd, bias=nmr)
                for half in range(2):
                    ptb = psum_s.tile([P, 512], BF16, name="ptb", tag="ps")
                    for q in range(4):
                        dcx = half * 4 + q
                        nc.tensor.transpose(ptb[:, q * P:(q + 1) * P],
                                            xnb[:, dcx * P:(dcx + 1) * P],
                                            ident_b)
                    dst = x1T[:, half * 4:(half + 1) * 4, tcx * P:(tcx + 1) * P]
                    srcv = ptb[:, :].rearrange("p (a b) -> p a b", a=4)
                    if half == 0:
                        nc.vector.tensor_copy(dst, srcv)
                    else:
                        nc.scalar.copy(dst, srcv)

            # prefetch the query-half x chunks
            xts = {}
            for tcx in (4, 5, 6, 7):
                xt = xw.tile([P, D], BF16, name="xt")
                nc.sync.dma_start(out=xt, in_=x_t[:, tcx, :])
                xts[tcx] = xt

            # R GEMM (PE works while LN1 runs on vector/scalar engines)
            with ExitStack() as phr:
                wpp = phr.enter_context(
                    tc.tile_pool(name="wpp", bufs=1, side="right"))
                wpos = wpp.tile([P, DC, D], F8)
                wp_ap = wpos_d[:].rearrange("(kc p) n -> p kc n", p=P)
                nc.sync.dma_start(out=wpos, in_=wp_ap)
                posT_sb = wpp.tile([P, DC, FULL], F8)
                nc.sync.dma_start(
                    out=posT_sb,
                    in_=posT_d[:].rearrange("(kc p) f -> p kc f", p=P))
                for n in range(DC):
                    rp = PB()
                    for fh in range(2):
                        for t in range(4):
                            nc.tensor.matmul(
                                rp[:, fh * 512:(fh + 1) * 512],
                                lhsT=wpos[:, 2 * t:2 * t + 2, n * P:(n + 1) * P],
                                rhs=posT_sb[:, 2 * t:2 * t + 2,
                                            fh * 512:(fh + 1) * 512],
                                start=(t == 0), stop=(t == 3), perf_mode=DR,
                                skip_group_check=True)
                    ro('v' if n % 2 else 's', rT[:, n, :], rp, 1.0 / WS,
                       bpos_sb[:, n:n + 1])

            for tcx in (4, 5, 6, 7):
                ln1_chunk(tcx, xts.pop(tcx))

            # Q GEMM on the just-normalized query chunks
            with ExitStack() as phq:
                wqp = phq.enter_context(
                    tc.tile_pool(name="wqp", bufs=1, side="right"))
                wq = wqp.tile([P, DC, D], F8)
                wq_ap = wq_d[:].rearrange("(kc p) n -> p kc n", p=P)
                nc.sync.dma_start(out=wq, in_=wq_ap)
                for np2 in range(DC // 2):
                    qp = PB()
                    for sub in range(2):
                        n = np2 * 2 + sub
                        for t in range(4):
                            nc.tensor.matmul(
                                qp[:, sub * 512:(sub + 1) * 512],
                                lhsT=wq[:, 2 * t:2 * t + 2, n * P:(n + 1) * P],
                                rhs=x1T[:, 2 * t:2 * t + 2, CUR:FULL],
                                start=(t == 0), stop=(t == 3), perf_mode=DR,
                                skip_group_check=True)
                    for sub in range(2):
                        n = np2 * 2 + sub
                        sl = qp[:, sub * 512:(sub + 1) * 512]
                        ro('v', quT[:, n, :], sl, 1.0 / WS, su_sb[:, n:n + 1])
                        ro('s', qvT[:, n, :], sl, 1.0 / WS, sv_sb[:, n:n + 1])

            for tcx in (0, 1, 2, 3):
                xt = xw.tile([P, D], BF16, name="xt")
                nc.sync.dma_start(out=xt, in_=x_t[:, tcx, :])
                ln1_chunk(tcx, xt)

        # rel-shift pads for all 16 scratch blocks (before any shifted read)
        for g4 in range(4):
            pad_ap = bass.AP(tensor=scr_all.tensor,
                             offset=scr_all.offset + g4 * 4 * SCRB + 1024,
                             ap=[[1536, P], [SCRB, 4], [1, 512]])
            nc.sync.dma_start(out=pad_ap, in_=padw4)

        # inpT loads + x8 fp8 conversion (needed at GRU1; overlaps attention)
        nc.sync.dma_start(
            out=inpT_f, in_=inpT_d[:].rearrange("(kc p) t -> p kc t", p=P))
        for n in range(DC):
            if n % 2 == 0:
                nc.vector.tensor_scalar_mul(inpT8[:, n, :], inpT_f[:, n, :], AS)
            else:
                nc.gpsimd.tensor_scalar_mul(inpT8[:, n, :], inpT_f[:, n, :], AS)

        with ExitStack() as ph:
            shw = ph.enter_context(tc.tile_pool(name="shw", bufs=3, side="left"))
            pbw = ph.enter_context(tc.tile_pool(name="pbw", bufs=3, side="left"))
            rw = ph.enter_context(tc.tile_pool(name="rw", bufs=2, side="left"))
            wpr = ph.enter_context(tc.tile_pool(name="wpr", bufs=1, side="left"))
            gwp1 = ph.enter_context(tc.tile_pool(name="gw1", bufs=2, side="left"))

            def head_aps(h):
                ch, rb = h // 2, (h % 2) * HEAD_DIM
                return (quT[rb:rb + HEAD_DIM, ch, :],
                        qvT[rb:rb + HEAD_DIM, ch, :],
                        kT[rb:rb + HEAD_DIM, ch, :],
                        rT[rb:rb + HEAD_DIM, ch, :], ch, rb)

            def emit_front(h):
                """pos scores -> pb4 -> scratch write -> shifted read"""
                _, qvh, _, rh, ch, rb = head_aps(h)
                b0 = (h % 4) * 4  # scratch block base (4-head rotation)
                pb4 = pbw.tile([P, TCC, 1024], F8, name="pb4")
                for ic in range(TCC):
                    pp = PB()
                    for jh in range(2):
                        nc.tensor.matmul(pp[:, jh * 512:(jh + 1) * 512],
                                         lhsT=qvh[:, ic * P:(ic + 1) * P],
                                         rhs=rh[:, jh * 512:(jh + 1) * 512],
                                         start=True, stop=True,
                                         skip_group_check=True)
                    if ic < 3:
                        nc.vector.tensor_copy(pb4[:, ic, :], pp)
                    else:
                        nc.scalar.copy(pb4[:, ic, :], pp)
                sw_ap = bass.AP(tensor=scr_all.tensor,
                                offset=scr_all.offset + b0 * SCRB,
                                ap=[[1536, P], [SCRB, TCC], [1, 1024]])
                nc.sync.dma_start(out=sw_ap, in_=pb4)
                shp4 = shw.tile([P, TCC, 1024], F8, name="shp4")
                sr_ap = bass.AP(tensor=scr_all.tensor,
                                offset=scr_all.offset + b0 * SCRB + 511,
                                ap=[[1535, P], [SCRB, TCC], [1, 1024]])
                nc.sync.dma_start(out=shp4, in_=sr_ap)
                return shp4

            # fronts 0..2 start their scratch roundtrips before K/V
            pend = {}
            for h in range(3):
                pend[h] = emit_front(h)

            # ---- K and V (nested pool; fills hide front latencies) ----
            with ExitStack() as phkv:
                wkvp = phkv.enter_context(
                    tc.tile_pool(name="wkvp", bufs=1, side="right"))
                wkv = wkvp.tile([P, DC, 2 * D], F8)
                wr_ap = wkv_d[:].rearrange("(kc p) n -> p kc n", p=P)
                for hf in range(2):
                    nc.sync.dma_start(out=wkv[:, hf * 4:(hf + 1) * 4, :],
                                      in_=wr_ap[:, hf * 4:(hf + 1) * 4, :])
                for i in range(DC):
                    # K chunk n=i: out [128n, 1024t]; psum = 32 * k_nat
                    kp = PB()
                    for th in range(2):
                        for t in range(4):
                            nc.tensor.matmul(
                                kp[:, th * 512:(th + 1) * 512],
                                lhsT=wkv[:, 2 * t:2 * t + 2, i * P:(i + 1) * P],
                                rhs=x1T[:, 2 * t:2 * t + 2,
                                        th * 512:(th + 1) * 512],
                                start=(t == 0), stop=(t == 3), perf_mode=DR,
                                skip_group_check=True)
                    ro('v' if i % 2 else 's', kT[:, i, :], kp, 1.0 / WS,
                       bkvK_sb[:, i:i + 1])
                    # V chunk t=i: out [128t, 1024f] -> v_ext strided
                    vp = PB()
                    for nh in range(2):
                        for k in range(4):
                            nc.tensor.matmul(
                                vp[:, nh * 512:(nh + 1) * 512],
                                lhsT=x1T[:, 2 * k:2 * k + 2, i * P:(i + 1) * P],
                                rhs=wkv[:, 2 * k:2 * k + 2,
                                        D + nh * 512:D + (nh + 1) * 512],
                                start=(k == 0), stop=False, perf_mode=DR,
                                skip_group_check=True)
                        nc.tensor.matmul(vp[:, nh * 512:(nh + 1) * 512],
                                         lhsT=ones_row,
                                         rhs=bkvV_sb[:, nh * 512:(nh + 1) * 512],
                                         start=False, stop=True,
                                         skip_group_check=True)
                    nc.scalar.activation(
                        out=v_ext[:, i, :, 0:64],
                        in_=vp[:, :].rearrange("p (a b) -> p a b", a=16),
                        func=Act.Copy, scale=1.0 / WS)

            esw = ph.enter_context(tc.tile_pool(name="esw", bufs=2, side="left"))
            etw = ph.enter_context(tc.tile_pool(name="etw", bufs=2, side="left"))
            avT, fr_avT = mk("avT", (P, DC, CUR), F8, "left")

            def emit_back(h, shp4):
                """content + shift-add + exp + XBAR + AV + normalize"""
                quh, _, kh, _, ch, rb = head_aps(h)
                es = esw.tile([P, TCC, 1024], BF16, name="es")
                for ic in range(TCC):
                    w = _wid(ic)
                    cp = PB()
                    nc.tensor.matmul(cp[:, 0:512],
                                     lhsT=quh[:, ic * P:(ic + 1) * P],
                                     rhs=kh[:, 0:512], start=True, stop=False,
                                     skip_group_check=True)
                    nc.tensor.matmul(cp[:, 512:w],
                                     lhsT=quh[:, ic * P:(ic + 1) * P],
                                     rhs=kh[:, 512:w], start=True, stop=False,
                                     skip_group_check=True)
                    nc.tensor.matmul(cp[:, 0:512], lhsT=ident_8,
                                     rhs=shp4[:, ic, 0:512], start=False,
                                     stop=False, skip_group_check=True)
                    nc.tensor.matmul(cp[:, 512:w], lhsT=ident_8,
                                     rhs=shp4[:, ic, 512:w], start=False,
                                     stop=True, skip_group_check=True)
                    nc.scalar.activation(out=es[:, ic, 0:w], in_=cp[:, 0:w],
                                         func=Act.Exp, scale=SCALE)
                esT = etw.tile([P, TCC, TCF, P], BF16, name="esT")
                for ic in range(TCC):
                    w = _wid(ic)
                    nc.sync.dma_start_transpose(esT[:, ic, 0:w // P, :],
                                                es[:, ic, 0:w])
                av = psum_s.tile([P, 512], F32, name="av", tag="ps")
                for jc in range(TCF):
                    ic0 = _qlo(jc) // P
                    nc.tensor.matmul(av[0:65, ic0 * P:512],
                                     lhsT=v_ext[:, jc, h, :],
                                     rhs=esT[:, ic0:TCC, jc, :],
                                     start=(jc == 0), stop=(jc == TCF - 1),
                                     skip_group_check=True)
                recip = rw.tile([1, 512], F32, name="recip")
                nc.vector.reciprocal(out=recip, in_=av[64:65, :])
                recipB = rw.tile([HEAD_DIM, 512], F32, name="recipB")
                nc.gpsimd.partition_broadcast(recipB, recip)
                nc.vector.tensor_mul(avT[rb:rb + HEAD_DIM, ch, :],
                                     av[0:HEAD_DIM, :], recipB)

            # back/front software pipeline with woven weight prefetch
            wproj = wpr.tile([P, DC, D], F8)
            g1pre = {}
            for h in range(HEAD_NUM):
                emit_back(h, pend.pop(h))
                if h + 3 < HEAD_NUM:
                    pend[h + 3] = emit_front(h + 3)
                if h == 6:
                    nc.sync.dma_start(
                        out=wproj,
                        in_=wproj_d[:].rearrange("(kc p) n -> p kc n", p=P))
                if h == 10:
                    for m in ("Wr", "Ur"):
                        w = gwp1.tile([P, DC, D], F8, name=f"g1_{m}", tag="gwt")
                        nc.sync.dma_start(
                            out=w,
                            in_=gw_d[(1, m)][:].rearrange("(kc p) n -> p kc n", p=P))
                        g1pre[m] = w
            fr_qvT(); fr_quT(); fr_rT(); fr_v(); fr_kT()

            o1T_f, fr_o1f = mk("o1T_f", (P, DC, CUR), F32, "right")
            o1T_b, fr_o1b = mk("o1T_b", (P, DC, CUR), BF16, "right")
            o1T8, fr_o18 = mk("o1T8", (P, DC, CUR), F8, "right")
            a1T, fr_a1T = mk("a1T", (P, DC, CUR), F8, "right")
            for np2 in range(DC // 2):
                pp = PB()
                for sub in range(2):
                    n = np2 * 2 + sub
                    sl = pp[:, sub * 512:(sub + 1) * 512]
                    for t in range(4):
                        nc.tensor.matmul(
                            sl, lhsT=wproj[:, 2 * t:2 * t + 2, n * P:(n + 1) * P],
                            rhs=avT[:, 2 * t:2 * t + 2, :],
                            start=(t == 0), stop=(t == 3), perf_mode=DR,
                            skip_group_check=True)
                    # psum = 256*(av@Wproj); a1T = 8*relu(av@Wproj + bproj)
                    ro('s', a1T[:, n, :], sl, 1.0 / WS, bproj8_sb[:, n:n + 1],
                       relu=True)
            fr_avT()

            # LN2 sums accumulate inside GRU1's output loop (s1 = sum o1,
            # s2 = sum o1^2, both [1,512] chains in one psum tile's banks)
            s1t = psum_s.tile([P, 512], F32, name="s1t", tag="ps")
            s2t = psum_s.tile([P, 512], F32, name="s2t", tag="ps")
            s1 = s1t[0:1, :]
            s2 = s2t[0:1, :]
            sqw = ph.enter_context(tc.tile_pool(name="sqw", bufs=2, side="left"))

            def post1(n):
                sq = sqw.tile([P, 512], BF16, name="sq")
                nc.vector.tensor_mul(sq, o1T_b[:, n, :], o1T_b[:, n, :])
                nc.tensor.matmul(s1, lhsT=ones_red, rhs=o1T_b[:, n, :],
                                 start=(n == 0), stop=(n == DC - 1),
                                 skip_group_check=True)
                nc.tensor.matmul(s2, lhsT=ones_red, rhs=sq,
                                 start=(n == 0), stop=(n == DC - 1),
                                 skip_group_check=True)

            _gru(nc, tc, ph, PB, gw_d, 1, a1T, inpT8, inpT_f, nbg1_sb,
                 o1T_f, o1T_b, o1T8, post_n=post1, gwp=gwp1, pre=g1pre)
        fr_a1T(); fr_x1T()

        # ========== Phase 5: LN2 (no affine) -> x2T8 (x8 fp8) ==========
        x2T8, fr_x2T = mk("x2T8", (P, DC, CUR), F8, "right")
        with ExitStack() as ph:
            lw = ph.enter_context(tc.tile_pool(name="lw", bufs=2, side="left"))
            mean = lw.tile([1, 512], F32, name="mean")
            nc.vector.tensor_scalar_mul(mean, s1, 1.0 / D)
            m2m = lw.tile([1, 512], F32, name="m2m")
            nc.vector.tensor_scalar_mul(m2m, s2, 1.0 / D)
            var = lw.tile([1, 512], F32, name="var")
            nc.vector.scalar_tensor_tensor(out=var, in0=mean, scalar=1.0,
                                           in1=mean, op0=AluOp.mult,
                                           op1=AluOp.mult)
            nc.vector.tensor_sub(var, m2m, var)
            # sd8 = sqrt((var+eps)/64) = sd/8 ; recip -> 8/sd
            sd8 = lw.tile([1, 512], F32, name="sd8")
            nc.scalar.activation(out=sd8, in_=var, func=Act.Sqrt,
                                 scale=1.0 / 64.0, bias=eps64_t)
            rstd8 = lw.tile([1, 512], F32, name="rstd8")
            nc.vector.reciprocal(out=rstd8, in_=sd8)
            meanB = lw.tile([P, 512], F32, name="meanB")
            nc.gpsimd.partition_broadcast(meanB, mean)
            rstd8B = lw.tile([P, 512], F32, name="rstd8B")
            nc.gpsimd.partition_broadcast(rstd8B, rstd8)
            meanB2 = bass.AP(tensor=meanB.tensor, offset=meanB.offset,
                             ap=[meanB[:].ap[0], [0, 2], [1, 512]])
            rstd8B2 = bass.AP(tensor=rstd8B.tensor, offset=rstd8B.offset,
                              ap=[rstd8B[:].ap[0], [0, 2], [1, 512]])
            for np2 in range(DC // 2):
                t1 = lw.tile([P, 2, 512], F32, name="t1")
                sl_i = o1T_f[:, 2 * np2:2 * np2 + 2, :]
                sl_o = x2T8[:, 2 * np2:2 * np2 + 2, :]
                if np2 % 2 == 0:
                    nc.vector.tensor_sub(t1, sl_i, meanB2)
                    nc.vector.tensor_mul(sl_o, t1, rstd8B2)
                else:
                    nc.gpsimd.tensor_sub(t1, sl_i, meanB2)
                    nc.gpsimd.tensor_mul(sl_o, t1, rstd8B2)

        # ========== Phase 6: MLP ========================================
        m2T, fr_m2T = mk("m2T", (P, DC, CUR), F8, "left")
        with ExitStack() as ph6:
            w1p = ph6.enter_context(tc.tile_pool(name="w1p", bufs=1, side="right"))
            m1w = ph6.enter_context(tc.tile_pool(name="m1w", bufs=1, side="right"))
            w2p = ph6.enter_context(tc.tile_pool(name="w2p", bufs=1, side="right"))
            w1 = w1p.tile([P, DC, HID], F8)
            w1_ap = w1_d[:].rearrange("(kc p) n -> p kc n", p=P)
            for hf in range(2):
                nc.sync.dma_start(out=w1[:, hf * 4:(hf + 1) * 4, :],
                                  in_=w1_ap[:, hf * 4:(hf + 1) * 4, :])
            w2 = w2p.tile([P, HC, D], F8)
            w2_ap = w2_d[:].rearrange("(kc p) n -> p kc n", p=P)
            for hf in range(2):
                nc.sync.dma_start(out=w2[:, hf * 16:(hf + 1) * 16, :],
                                  in_=w2_ap[:, hf * 16:(hf + 1) * 16, :])
            m1T = m1w.tile([P, HC, 512], F8)
            for np2 in range(HC // 2):
                pp = PB()
                for sub in range(2):
                    n = np2 * 2 + sub
                    sl = pp[:, sub * 512:(sub + 1) * 512]
                    for t in range(4):
                        nc.tensor.matmul(
                            sl, lhsT=w1[:, 2 * t:2 * t + 2, n * P:(n + 1) * P],
                            rhs=x2T8[:, 2 * t:2 * t + 2, :],
                            start=(t == 0), stop=(t == 3), perf_mode=DR,
                            skip_group_check=True)
                    # W1 stored x8, x2 x8 -> psum = 64*(x2@W1f);
                    # m1T = 64*relu(x2@W1f + b1f) via one (add,max) op
                    if n % 2 == 0:
                        nc.vector.tensor_scalar(out=m1T[:, n, :], in0=sl,
                                                scalar1=b1f64_sb[:, n:n + 1],
                                                scalar2=0.0, op0=AluOp.add,
                                                op1=AluOp.max)
                    else:
                        nc.scalar.activation(out=m1T[:, n, :], in_=sl,
                                             func=Act.Relu, scale=1.0,
                                             bias=b1f64_sb[:, n:n + 1])
            for np2 in range(DC // 2):
                pp = PB()
                for sub in range(2):
                    n = np2 * 2 + sub
                    sl = pp[:, sub * 512:(sub + 1) * 512]
                    for t in range(16):
                        nc.tensor.matmul(
                            sl, lhsT=w2[:, 2 * t:2 * t + 2, n * P:(n + 1) * P],
                            rhs=m1T[:, 2 * t:2 * t + 2, :],
                            start=(t == 0), stop=(t == 15), perf_mode=DR,
                            skip_group_check=True)
                    # psum = 64*32*(m1@W2) = 2048x ; m2T = 8*relu(...)
                    ro('s', m2T[:, n, :], sl, 1.0 / 256.0, b2_8_sb[:, n:n + 1],
                       relu=True)
        fr_x2T()

        # ========== Phase 7: GRU2 =======================================
        o2T_f, fr_o2 = mk("o2T_f", (P, DC, CUR), F32, "left")
        with ExitStack() as ph:
            ow = ph.enter_context(tc.tile_pool(name="ow", bufs=2, side="left"))

            def post2(n):
                # transpose this output feature chunk and stream it out
                pt = PS()
                for q in range(TCC):
                    nc.tensor.transpose(pt[:, q * P:(q + 1) * P],
                                        o2T_f[:, n, q * P:(q + 1) * P],
                                        ident_f)
                on = ow.tile([P, TCC, P], F32, name="on")
                if n % 2 == 0:
                    nc.vector.tensor_copy(on, pt[:, :].rearrange(
                        "p (a b) -> p a b", a=TCC))
                else:
                    nc.scalar.copy(on, pt[:, :].rearrange(
                        "p (a b) -> p a b", a=TCC))
                o_ap = bass.AP(tensor=out_d.tensor,
                               offset=out_d.offset + n * P,
                               ap=[[D, P], [P * D, TCC], [1, P]])
                nc.sync.dma_start(out=o_ap, in_=on)

            _gru(nc, tc, ph, PB, gw_d, 2, m2T, o1T8, o1T_f, nbg2_sb,
                 o2T_f, None, None, post_n=post2)
        fr_o18(); fr_o1b(); fr_o1f(); fr_inp8(); fr_inpf()

        fr_o2(); fr_m2T()


def _gru(nc, tc, ph, PB, gw_d, g, yT8, xT8, xT_f, nbg_sb, oT_f, oT_b, oT8,
         post_n=None, gwp=None, pre=None):
    """GRU gate: yT8/xT8 fp8 at 8x scale; psum = 256*(y@W + x@U)."""
    if gwp is None:
        gwp = ph.enter_context(tc.tile_pool(name=f"gw{g}", bufs=3, side="left"))
    gtmp = ph.enter_context(tc.tile_pool(name=f"gt{g}", bufs=2, side="left"))
    gper = ph.enter_context(tc.tile_pool(name=f"gp{g}", bufs=1, side="left"))
    pre = pre or {}

    def loadw(m):
        if m in pre:
            return pre[m]
        w = gwp.tile([P, DC, D], F8, name=f"gwt_{m}", tag="gwt")
        ap = gw_d[(g, m)][:].rearrange("(kc p) n -> p kc n", p=P)
        nc.sync.dma_start(out=w, in_=ap)
        return w

    def gate_psum(ps, w, u, n, rhs_x):
        for t in range(4):
            nc.tensor.matmul(ps, lhsT=w[:, 2 * t:2 * t + 2, n * P:(n + 1) * P],
                             rhs=yT8[:, 2 * t:2 * t + 2, :],
                             start=(t == 0), stop=False, perf_mode=DR,
                             skip_group_check=True)
        for t in range(4):
            nc.tensor.matmul(ps, lhsT=u[:, 2 * t:2 * t + 2, n * P:(n + 1) * P],
                             rhs=rhs_x[:, 2 * t:2 * t + 2, :],
                             start=False, stop=(t == 3), perf_mode=DR,
                             skip_group_check=True)

    wr, ur = loadw("Wr"), loadw("Ur")
    rx8 = gper.tile([P, DC, 512], F8, name="rx8")
    for np2 in range(DC // 2):
        pp = PB()
        for sub in range(2):
            n = np2 * 2 + sub
            sl = pp[:, sub * 512:(sub + 1) * 512]
            gate_psum(sl, wr, ur, n, xT8)
            rr = gtmp.tile([P, 512], F32, name="rr")
            nc.scalar.activation(out=rr, in_=sl, func=Act.Sigmoid, scale=RS)
            # rx8 = (8*r) * x
            nc.vector.scalar_tensor_tensor(out=rx8[:, n, :], in0=rr, scalar=AS,
                                           in1=xT_f[:, n, :], op0=AluOp.mult,
                                           op1=AluOp.mult)
    wz, uz = loadw("Wz"), loadw("Uz")
    zt = gper.tile([P, DC, 512], BF16, name="zt")
    for np2 in range(DC // 2):
        pp = PB()
        for sub in range(2):
            n = np2 * 2 + sub
            sl = pp[:, sub * 512:(sub + 1) * 512]
            gate_psum(sl, wz, uz, n, xT8)
            nc.scalar.activation(out=zt[:, n, :], in_=sl, func=Act.Sigmoid,
                                 scale=RS, bias=nbg_sb[:, n:n + 1])
    wg, ug = loadw("Wg"), loadw("Ug")
    for np2 in range(DC // 2):
        pp = PB()
        for sub in range(2):
            n = np2 * 2 + sub
            sl = pp[:, sub * 512:(sub + 1) * 512]
            gate_psum(sl, wg, ug, n, rx8)
            ht = gtmp.tile([P, 512], BF16, name="ht")
            nc.scalar.activation(out=ht, in_=sl, func=Act.Tanh, scale=RS)
            d1 = gtmp.tile([P, 512], BF16, name="d1")
            nc.gpsimd.tensor_sub(d1, ht, xT_f[:, n, :])
            zd = gtmp.tile([P, 512], BF16, name="zd")
            nc.vector.tensor_mul(zd, d1, zt[:, n, :])
            nc.vector.tensor_add(oT_f[:, n, :], zd, xT_f[:, n, :])
            if oT_b is not None:
                nc.gpsimd.tensor_copy(oT_b[:, n, :], oT_f[:, n, :])
            if oT8 is not None:
                nc.gpsimd.tensor_scalar_mul(oT8[:, n, :], oT_f[:, n, :], AS)
            if post_n is not None:
                post_n(n)


_NC_CACHE = {}


def _get_nc():
    if "nc" not in _NC_CACHE:
        _NC_CACHE["nc"] = _build()
    return _NC_CACHE["nc"]


def _chunk_t(vec):
    n = vec.shape[0] // P
    return np.ascontiguousarray(vec.reshape(n, P).T.astype(np.float32))


def _f8(x, s=WS):
    return np.asarray(np.asarray(x, np.float32) * s, NF8)


def _prep(inputs):
    f32 = np.float32
    inp = np.asarray(inputs["inputs"], f32)
    mem = np.asarray(inputs["memory"], f32)
    pos = np.asarray(inputs["pos_embedding"], f32)[:, 0, :]

    g1 = np.asarray(inputs["ln1_g"], f32)
    b1 = np.asarray(inputs["ln1_b"], f32)
    g2 = np.asarray(inputs["ln2_g"], f32)
    b2 = np.asarray(inputs["ln2_b"], f32)
    Wkv = np.asarray(inputs["Wkv"], f32)
    Wq = np.asarray(inputs["Wq"], f32)
    W1 = np.asarray(inputs["mlp_W1"], f32)

    Wkv_f = g1[:, None] * Wkv
    bkv_f = b1 @ Wkv + np.asarray(inputs["bkv"], f32)
    Wq_f = g1[:, None] * Wq
    bq_f = b1 @ Wq + np.asarray(inputs["bq"], f32)
    W1_f = g2[:, None] * W1
    b1_f = b2 @ W1 + np.asarray(inputs["mlp_b1"], f32)

    u_flat = np.asarray(inputs["u"], f32).reshape(-1)
    v_flat = np.asarray(inputs["v"], f32).reshape(-1)

    shared = {
        "posT": np.ascontiguousarray(pos.T).astype(NF8),
        "Wkv": _f8(Wkv_f), "Wq": _f8(Wq_f), "Wpos": _f8(inputs["Wpos"]),
        "Wproj": _f8(inputs["Wproj"]),
        "mlp_W1": _f8(W1_f, 8.0), "mlp_W2": _f8(inputs["mlp_W2"]),
        "bkvV32_row": (bkv_f[D:2 * D] * WS).reshape(1, D).astype(NBF),
        "biases_t": np.concatenate([
            _chunk_t(bkv_f[0:D]),
            _chunk_t(bq_f + u_flat),
            _chunk_t(bq_f + v_flat),
            _chunk_t(np.asarray(inputs["bpos"], f32)),
            _chunk_t(np.asarray(inputs["bproj"], f32) * AS),
            _chunk_t(np.asarray(inputs["mlp_b2"], f32) * AS),
            _chunk_t(-np.asarray(inputs["g1_bg"], f32)),
            _chunk_t(-np.asarray(inputs["g2_bg"], f32)),
            _chunk_t(b1_f * 64.0),
        ], axis=1),
    }
    for g in (1, 2):
        for m in ("Wr", "Ur", "Wz", "Uz", "Wg", "Ug"):
            shared[f"g{g}_{m}"] = _f8(inputs[f"g{g}_{m}"])

    in_maps = []
    for b in range(BS):
        im = dict(shared)
        im["x_full"] = np.ascontiguousarray(
            np.concatenate([mem[:, b, :], inp[:, b, :]], axis=0)).astype(NBF)
        im["inpT"] = np.ascontiguousarray(inp[:, b, :].T)
        in_maps.append(im)
    return in_maps


def kernel(**inputs):
    nc = _get_nc()
    in_maps = _prep(inputs)
    res = run_bass_kernel_spmd(nc, in_maps, core_ids=list(range(BS)))
    out = np.stack([res.results[b]["out"] for b in range(BS)], axis=1)
    return np.ascontiguousarray(out.astype(np.float32))


if __name__ == "__main__":
    _get_nc()
    print("build+compile OK")


# revision 61
# speedup vs baseline: 1.0713x; 1.0004x over previous
"""GTrXL layer (TransformerXL attention + GRU gating) on 8 TRN2 NeuronCores.

Sharding: pure data-parallel over batch (BS=8 -> 1 batch element per core).
No collectives. Per-core Bass/Tile kernel computes the full layer for its
batch element.

v2: fp8(e4m3) DoubleRow matmuls for all dense GEMMs (K=256 per instruction
at 0.5 cycles/row = 4x bf16 PE throughput), attention scores in bf16 with
the rel-shift DMA trick in fp8, XBAR dma-transpose for the softmax
transposition, an extended-V matmul that produces the softmax denominator
for free, and LN affine folding into the consuming weights (host-side).

Scales (host pre-scales; exact powers of two, no extra rounding):
  - fp8 weights stored x32
  - fp8 activations feeding GRU/MLP/proj matmuls stored x8
  - every PSUM readout rescales back to natural units.
Scores/K/V/Q/R stay natural-scale bf16; the rel-shift pad is -240 (fp8 min)
so exp((content-240)*0.125) == 0 covers the causal mask for free.
"""

import sys

if '/opt/trn_rl_repo' not in sys.path:
    sys.path.insert(0, '/opt/trn_rl_repo')

import numpy as np
import ml_dtypes

import concourse.bass as bass
import concourse.tile as tile
from concourse import bacc, mybir
from concourse.bass_utils import run_bass_kernel_spmd
from concourse.masks import make_identity

F8 = mybir.dt.float8e4
BF16 = mybir.dt.bfloat16
F32 = mybir.dt.float32
NF8 = ml_dtypes.float8_e4m3
NBF = ml_dtypes.bfloat16

HEAD_NUM, HEAD_DIM = 16, 64
D, HID = 1024, 4096
CUR, PREV, BS = 512, 512, 8
FULL = CUR + PREV
EPS = 1e-5
SCALE = 1.0 / (HEAD_DIM ** 0.5)
P = 128
DC = D // P          # 8 feature chunks
HC = HID // P        # 32 hidden chunks
TCF = FULL // P      # 8 full-token chunks
TCC = CUR // P       # 4 query-token chunks
NEGPAD = -240.0      # fp8 e4m3 most-negative finite
WS = 32.0            # weight scale
AS = 8.0             # activation scale
RS = 1.0 / (WS * AS)

AluOp = mybir.AluOpType
Act = mybir.ActivationFunctionType
DR = mybir.MatmulPerfMode.DoubleRow


def _dram_in(dram, name, shape, dtype):
    return dram.tile(list(shape), dtype, kind="ExternalInput", name=name,
                     uniquify=False)


def _build():
    nc = bacc.Bacc("TRN2", target_bir_lowering=False)
    with tile.TileContext(nc) as tc:
        _emit(nc, tc)
    nc.compile()
    return nc


def _wid(ic):
    """causal key width for query chunk ic (keys j <= i + PREV)"""
    return (ic + 5) * P


def _qlo(jc):
    """first valid query row for key chunk jc"""
    return max(0, (jc - 4) * P)


def _emit(nc, tc):
    from contextlib import ExitStack

    with ExitStack() as root:
        dram = root.enter_context(tc.tile_pool(name="io", bufs=1, space="DRAM"))

        # ---------------- DRAM I/O ----------------
        x_full = _dram_in(dram, "x_full", (FULL, D), BF16)
        inpT_d = _dram_in(dram, "inpT", (D, CUR), F32)
        posT_d = _dram_in(dram, "posT", (D, FULL), F8)

        wkv_d = _dram_in(dram, "Wkv", (D, 2 * D), F8)
        wq_d = _dram_in(dram, "Wq", (D, D), F8)
        wpos_d = _dram_in(dram, "Wpos", (D, D), F8)
        wproj_d = _dram_in(dram, "Wproj", (D, D), F8)
        gw_d = {}
        for g in (1, 2):
            for m in ("Wr", "Ur", "Wz", "Uz", "Wg", "Ug"):
                gw_d[(g, m)] = _dram_in(dram, f"g{g}_{m}", (D, D), F8)
        w1_d = _dram_in(dram, "mlp_W1", (D, HID), F8)
        w2_d = _dram_in(dram, "mlp_W2", (HID, D), F8)

        biases_d = _dram_in(dram, "biases_t", (P, 96), F32)
        bkvV_d = _dram_in(dram, "bkvV32_row", (1, D), BF16)

        out_d = dram.tile([CUR, D], F32, kind="ExternalOutput", name="out",
                          uniquify=False)

        n_scr = 16
        SCRB = P * 1536  # elements per scratch block
        scr_all = dram.tile([n_scr, P, 1536], F8, name="scr_all")
        scr = [scr_all[s] for s in range(n_scr)]

        # ---------------- constants ----------------
        const = root.enter_context(tc.tile_pool(name="const", bufs=1))
        ident_f = const.tile([P, P], F32)
        make_identity(nc, ident_f)
        ident_8 = const.tile([P, P], F8)
        make_identity(nc, ident_8)
        ident_b = const.tile([P, P], BF16)
        make_identity(nc, ident_b)
        ones_row = const.tile([1, P], BF16)
        nc.vector.memset(ones_row, 1.0)
        ones_red = const.tile([P, 1], BF16)
        nc.vector.memset(ones_red, 1.0)
        eps_t = const.tile([P, 1], F32)
        nc.vector.memset(eps_t, EPS)
        eps64_t = const.tile([1, 1], F32)
        nc.vector.memset(eps64_t, EPS / 64.0)

        def cload(name, dref, shape, dtype=F32):
            t = const.tile(list(shape), dtype, name=name)
            nc.sync.dma_start(out=t, in_=dref[:])
            return t

        biases_sb = cload("biases_sb", biases_d, (P, 96))
        bkvV_sb = cload("bkvV_sb", bkvV_d, (1, D), BF16)
        bkvK_sb = biases_sb[:, 0:8]
        su_sb = biases_sb[:, 8:16]
        sv_sb = biases_sb[:, 16:24]
        bpos_sb = biases_sb[:, 24:32]
        bproj8_sb = biases_sb[:, 32:40]
        b2_8_sb = biases_sb[:, 40:48]
        nbg1_sb = biases_sb[:, 48:56]
        nbg2_sb = biases_sb[:, 56:64]
        b1f64_sb = biases_sb[:, 64:96]

        padw4 = const.tile([P, 4, 512], F8)
        nc.vector.memset(padw4, NEGPAD)

        # psum pools: big (scores) 3x 2 banks, small 2x 1 bank = 8 banks
        psum_b = root.enter_context(tc.tile_pool(name="psum_b", bufs=3, space="PSUM"))
        psum_s = root.enter_context(tc.tile_pool(name="psum_s", bufs=2, space="PSUM"))

        def PB():
            return psum_b.tile([P, 1024], F32, name="pbig", tag="pbig")

        def PS():
            return psum_s.tile([P, 512], F32, name="ps", tag="ps")

        def mk(name, shape, dtype, side):
            t, fr = tc.tile(list(shape), dtype, name=name, side=side)
            return t, fr

        # ---- engine-cycled psum readout: (ps * scale) + bias -> out ----
        def ro(eng, out, ps, scale, bias_ap, relu=False):
            if relu:
                nc.scalar.activation(out=out, in_=ps, func=Act.Relu,
                                     scale=scale, bias=bias_ap)
            elif eng == 's':
                nc.scalar.activation(out=out, in_=ps, func=Act.Identity,
                                     scale=scale, bias=bias_ap)
            elif eng == 'v':
                nc.vector.tensor_scalar(out=out, in0=ps, scalar1=scale,
                                        scalar2=bias_ap, op0=AluOp.mult,
                                        op1=AluOp.add)
            else:
                nc.gpsimd.tensor_scalar(out=out, in0=ps, scalar1=scale,
                                        scalar2=bias_ap, op0=AluOp.mult,
                                        op1=AluOp.add)

        # ========== Phase 1+2a: R first (LN1-independent), LN1 split ====
        # R = Wpos @ posT needs no LN1 output, so the PE computes it while
        # the vector/scalar engines run LN1. Query-token chunks (4..7) are
        # normalized first so the Q GEMM starts early too.
        inpT_f, fr_inpf = mk("inpT_f", (P, DC, CUR), F32, "right")
        inpT8, fr_inp8 = mk("inpT8", (P, DC, CUR), F8, "right")
        x1T, fr_x1T = mk("x1T", (P, DC, FULL), F8, "left")
        kT, fr_kT = mk("kT", (P, DC, FULL), BF16, "right")
        v_ext, fr_v = mk("v_ext", (P, TCF, HEAD_NUM, 65), BF16, "right")
        rT, fr_rT = mk("rT", (P, DC, FULL), BF16, "right")
        quT, fr_quT = mk("quT", (P, DC, CUR), BF16, "right")
        qvT, fr_qvT = mk("qvT", (P, DC, CUR), BF16, "right")
        nc.vector.memset(v_ext[:, :, :, 64:65], 0.125)

        with ExitStack() as ph:
            xw = ph.enter_context(tc.tile_pool(name="xw", bufs=4, side="right"))
            st = ph.enter_context(tc.tile_pool(name="st", bufs=3, side="right"))
            x_t = x_full[:].rearrange("(tc p) d -> p tc d", p=P)

            def ln1_chunk(tcx, xt):
                stats = st.tile([P, 2, 6], F32, name="stats")
                nc.vector.bn_stats(out=stats[:, 0, :], in_=xt[:, 0:512])
                nc.vector.bn_stats(out=stats[:, 1, :], in_=xt[:, 512:1024])
                mv = st.tile([P, 2], F32, name="mv")
                nc.vector.bn_aggr(out=mv, in_=stats)
                sd = st.tile([P, 1], F32, name="sd")
                nc.scalar.activation(out=sd, in_=mv[:, 1:2], func=Act.Sqrt,
                                     bias=eps_t)
                rstd = st.tile([P, 1], F32, name="rstd")
                nc.vector.reciprocal(out=rstd, in_=sd)
                nmr = st.tile([P, 1], F32, name="nmr")
                nc.vector.scalar_tensor_tensor(out=nmr, in0=mv[:, 0:1],
                                               scalar=-1.0, in1=rstd,
                                               op0=AluOp.mult, op1=AluOp.mult)
                xnb = xw.tile([P, D], BF16, name="xnb")
                nc.gpsimd.tensor_scalar(out=xnb, in0=xt, scalar1=rstd,
                                        scalar2=nmr, op0=AluOp.mult,
                                        op1=AluOp.add)
                for half in range(2):
                    ptb = psum_s.tile([P, 512], BF16, name="ptb", tag="ps")
                    for q in range(4):
                        dcx = half * 4 + q
                        nc.tensor.transpose(ptb[:, q * P:(q + 1) * P],
                                            xnb[:, dcx * P:(dcx + 1) * P],
                                            ident_b)
                    dst = x1T[:, half * 4:(half + 1) * 4, tcx * P:(tcx + 1) * P]
                    srcv = ptb[:, :].rearrange("p (a b) -> p a b", a=4)
                    nc.scalar.copy(dst, srcv)

            # prefetch the query-half x chunks
            xts = {}
            for tcx in (4, 5, 6, 7):
                xt = xw.tile([P, D], BF16, name="xt")
                nc.sync.dma_start(out=xt, in_=x_t[:, tcx, :])
                xts[tcx] = xt

            # R GEMM (PE works while LN1 runs on vector/scalar engines)
            with ExitStack() as phr:
                wpp = phr.enter_context(
                    tc.tile_pool(name="wpp", bufs=1, side="right"))
                wpos = wpp.tile([P, DC, D], F8)
                wp_ap = wpos_d[:].rearrange("(kc p) n -> p kc n", p=P)
                nc.sync.dma_start(out=wpos, in_=wp_ap)
                posT_sb = wpp.tile([P, DC, FULL], F8)
                nc.sync.dma_start(
                    out=posT_sb,
                    in_=posT_d[:].rearrange("(kc p) f -> p kc f", p=P))
                for n in range(DC):
                    rp = PB()
                    for fh in range(2):
                        for t in range(4):
                            nc.tensor.matmul(
                                rp[:, fh * 512:(fh + 1) * 512],
                                lhsT=wpos[:, 2 * t:2 * t + 2, n * P:(n + 1) * P],
                                rhs=posT_sb[:, 2 * t:2 * t + 2,
                                            fh * 512:(fh + 1) * 512],
                                start=(t == 0), stop=(t == 3), perf_mode=DR,
                                skip_group_check=True)
                    ro('v' if n % 2 else 's', rT[:, n, :], rp, 1.0 / WS,
                       bpos_sb[:, n:n + 1])

            for tcx in (4, 5, 6, 7):
                ln1_chunk(tcx, xts.pop(tcx))

            # Q GEMM on the just-normalized query chunks
            with ExitStack() as phq:
                wqp = phq.enter_context(
                    tc.tile_pool(name="wqp", bufs=1, side="right"))
                wq = wqp.tile([P, DC, D], F8)
                wq_ap = wq_d[:].rearrange("(kc p) n -> p kc n", p=P)
                nc.sync.dma_start(out=wq, in_=wq_ap)
                for np2 in range(DC // 2):
                    qp = PB()
                    for sub in range(2):
                        n = np2 * 2 + sub
                        for t in range(4):
                            nc.tensor.matmul(
                                qp[:, sub * 512:(sub + 1) * 512],
                                lhsT=wq[:, 2 * t:2 * t + 2, n * P:(n + 1) * P],
                                rhs=x1T[:, 2 * t:2 * t + 2, CUR:FULL],
                                start=(t == 0), stop=(t == 3), perf_mode=DR,
                                skip_group_check=True)
                    for sub in range(2):
                        n = np2 * 2 + sub
                        sl = qp[:, sub * 512:(sub + 1) * 512]
                        ro('v', quT[:, n, :], sl, 1.0 / WS, su_sb[:, n:n + 1])
                        ro('s', qvT[:, n, :], sl, 1.0 / WS, sv_sb[:, n:n + 1])

            for tcx in (0, 1, 2, 3):
                xt = xw.tile([P, D], BF16, name="xt")
                nc.sync.dma_start(out=xt, in_=x_t[:, tcx, :])
                ln1_chunk(tcx, xt)

        # rel-shift pads for all 16 scratch blocks (before any shifted read)
        for g4 in range(4):
            pad_ap = bass.AP(tensor=scr_all.tensor,
                             offset=scr_all.offset + g4 * 4 * SCRB + 1024,
                             ap=[[1536, P], [SCRB, 4], [1, 512]])
            nc.sync.dma_start(out=pad_ap, in_=padw4)

        # inpT loads + x8 fp8 conversion (needed at GRU1; overlaps attention)
        nc.sync.dma_start(
            out=inpT_f, in_=inpT_d[:].rearrange("(kc p) t -> p kc t", p=P))
        for n in range(DC):
            if n % 2 == 0:
                nc.vector.tensor_scalar_mul(inpT8[:, n, :], inpT_f[:, n, :], AS)
            else:
                nc.gpsimd.tensor_scalar_mul(inpT8[:, n, :], inpT_f[:, n, :], AS)

        with ExitStack() as ph:
            shw = ph.enter_context(tc.tile_pool(name="shw", bufs=3, side="left"))
            pbw = ph.enter_context(tc.tile_pool(name="pbw", bufs=3, side="left"))
            rw = ph.enter_context(tc.tile_pool(name="rw", bufs=2, side="left"))
            wpr = ph.enter_context(tc.tile_pool(name="wpr", bufs=1, side="left"))
            gwp1 = ph.enter_context(tc.tile_pool(name="gw1", bufs=2, side="left"))

            def head_aps(h):
                ch, rb = h // 2, (h % 2) * HEAD_DIM
                return (quT[rb:rb + HEAD_DIM, ch, :],
                        qvT[rb:rb + HEAD_DIM, ch, :],
                        kT[rb:rb + HEAD_DIM, ch, :],
                        rT[rb:rb + HEAD_DIM, ch, :], ch, rb)

            def emit_front(h):
                """pos scores -> pb4 -> scratch write -> shifted read"""
                _, qvh, _, rh, ch, rb = head_aps(h)
                b0 = (h % 4) * 4  # scratch block base (4-head rotation)
                pb4 = pbw.tile([P, TCC, 1024], F8, name="pb4")
                for ic in range(TCC):
                    pp = PB()
                    for jh in range(2):
                        nc.tensor.matmul(pp[:, jh * 512:(jh + 1) * 512],
                                         lhsT=qvh[:, ic * P:(ic + 1) * P],
                                         rhs=rh[:, jh * 512:(jh + 1) * 512],
                                         start=True, stop=True,
                                         skip_group_check=True)
                    if ic < 3:
                        nc.vector.tensor_copy(pb4[:, ic, :], pp)
                    else:
                        nc.scalar.copy(pb4[:, ic, :], pp)
                sw_ap = bass.AP(tensor=scr_all.tensor,
                                offset=scr_all.offset + b0 * SCRB,
                                ap=[[1536, P], [SCRB, TCC], [1, 1024]])
                nc.sync.dma_start(out=sw_ap, in_=pb4)
                shp4 = shw.tile([P, TCC, 1024], F8, name="shp4")
                sr_ap = bass.AP(tensor=scr_all.tensor,
                                offset=scr_all.offset + b0 * SCRB + 511,
                                ap=[[1535, P], [SCRB, TCC], [1, 1024]])
                nc.sync.dma_start(out=shp4, in_=sr_ap)
                return shp4

            # fronts 0..2 start their scratch roundtrips before K/V
            pend = {}
            for h in range(3):
                pend[h] = emit_front(h)

            # ---- K and V (nested pool; fills hide front latencies) ----
            with ExitStack() as phkv:
                wkvp = phkv.enter_context(
                    tc.tile_pool(name="wkvp", bufs=1, side="right"))
                wkv = wkvp.tile([P, DC, 2 * D], F8)
                wr_ap = wkv_d[:].rearrange("(kc p) n -> p kc n", p=P)
                for hf in range(2):
                    nc.sync.dma_start(out=wkv[:, hf * 4:(hf + 1) * 4, :],
                                      in_=wr_ap[:, hf * 4:(hf + 1) * 4, :])
                for i in range(DC):
                    # K chunk n=i: out [128n, 1024t]; psum = 32 * k_nat
                    kp = PB()
                    for th in range(2):
                        for t in range(4):
                            nc.tensor.matmul(
                                kp[:, th * 512:(th + 1) * 512],
                                lhsT=wkv[:, 2 * t:2 * t + 2, i * P:(i + 1) * P],
                                rhs=x1T[:, 2 * t:2 * t + 2,
                                        th * 512:(th + 1) * 512],
                                start=(t == 0), stop=(t == 3), perf_mode=DR,
                                skip_group_check=True)
                    ro('v' if i % 2 else 's', kT[:, i, :], kp, 1.0 / WS,
                       bkvK_sb[:, i:i + 1])
                    # V chunk t=i: out [128t, 1024f] -> v_ext strided
                    vp = PB()
                    for nh in range(2):
                        for k in range(4):
                            nc.tensor.matmul(
                                vp[:, nh * 512:(nh + 1) * 512],
                                lhsT=x1T[:, 2 * k:2 * k + 2, i * P:(i + 1) * P],
                                rhs=wkv[:, 2 * k:2 * k + 2,
                                        D + nh * 512:D + (nh + 1) * 512],
                                start=(k == 0), stop=False, perf_mode=DR,
                                skip_group_check=True)
                        nc.tensor.matmul(vp[:, nh * 512:(nh + 1) * 512],
                                         lhsT=ones_row,
                                         rhs=bkvV_sb[:, nh * 512:(nh + 1) * 512],
                                         start=False, stop=True,
                                         skip_group_check=True)
                    nc.scalar.activation(
                        out=v_ext[:, i, :, 0:64],
                        in_=vp[:, :].rearrange("p (a b) -> p a b", a=16),
                        func=Act.Copy, scale=1.0 / WS)

            esw = ph.enter_context(tc.tile_pool(name="esw", bufs=2, side="left"))
            etw = ph.enter_context(tc.tile_pool(name="etw", bufs=2, side="left"))
            avT, fr_avT = mk("avT", (P, DC, CUR), F8, "left")

            def emit_back(h, shp4):
                """content + shift-add + exp + XBAR + AV + normalize"""
                quh, _, kh, _, ch, rb = head_aps(h)
                es = esw.tile([P, TCC, 1024], BF16, name="es")
                for ic in range(TCC):
                    w = _wid(ic)
                    cp = PB()
                    nc.tensor.matmul(cp[:, 0:512],
                                     lhsT=quh[:, ic * P:(ic + 1) * P],
                                     rhs=kh[:, 0:512], start=True, stop=False,
                                     skip_group_check=True)
                    nc.tensor.matmul(cp[:, 512:w],
                                     lhsT=quh[:, ic * P:(ic + 1) * P],
                                     rhs=kh[:, 512:w], start=True, stop=False,
                                     skip_group_check=True)
                    nc.tensor.matmul(cp[:, 0:512], lhsT=ident_8,
                                     rhs=shp4[:, ic, 0:512], start=False,
                                     stop=False, skip_group_check=True)
                    nc.tensor.matmul(cp[:, 512:w], lhsT=ident_8,
                                     rhs=shp4[:, ic, 512:w], start=False,
                                     stop=True, skip_group_check=True)
                    nc.scalar.activation(out=es[:, ic, 0:w], in_=cp[:, 0:w],
                                         func=Act.Exp, scale=SCALE)
                esT = etw.tile([P, TCC, TCF, P], BF16, name="esT")
                for ic in range(TCC):
                    w = _wid(ic)
                    nc.sync.dma_start_transpose(esT[:, ic, 0:w // P, :],
                                                es[:, ic, 0:w])
                av = psum_s.tile([P, 512], F32, name="av", tag="ps")
                for jc in range(TCF):
                    ic0 = _qlo(jc) // P
                    nc.tensor.matmul(av[0:65, ic0 * P:512],
                                     lhsT=v_ext[:, jc, h, :],
                                     rhs=esT[:, ic0:TCC, jc, :],
                                     start=(jc == 0), stop=(jc == TCF - 1),
                                     skip_group_check=True)
                recip = rw.tile([1, 512], F32, name="recip")
                nc.vector.reciprocal(out=recip, in_=av[64:65, :])
                recipB = rw.tile([HEAD_DIM, 512], F32, name="recipB")
                nc.gpsimd.partition_broadcast(recipB, recip)
                nc.vector.tensor_mul(avT[rb:rb + HEAD_DIM, ch, :],
                                     av[0:HEAD_DIM, :], recipB)

            # back/front software pipeline with woven weight prefetch
            wproj = wpr.tile([P, DC, D], F8)
            g1pre = {}
            for h in range(HEAD_NUM):
                emit_back(h, pend.pop(h))
                if h + 3 < HEAD_NUM:
                    pend[h + 3] = emit_front(h + 3)
                if h == 6:
                    nc.sync.dma_start(
                        out=wproj,
                        in_=wproj_d[:].rearrange("(kc p) n -> p kc n", p=P))
                if h == 10:
                    for m in ("Wr", "Ur"):
                        w = gwp1.tile([P, DC, D], F8, name=f"g1_{m}", tag="gwt")
                        nc.sync.dma_start(
                            out=w,
                            in_=gw_d[(1, m)][:].rearrange("(kc p) n -> p kc n", p=P))
                        g1pre[m] = w
            fr_qvT(); fr_quT(); fr_rT(); fr_v(); fr_kT()

            o1T_f, fr_o1f = mk("o1T_f", (P, DC, CUR), F32, "right")
            o1T_b, fr_o1b = mk("o1T_b", (P, DC, CUR), BF16, "right")
            o1T8, fr_o18 = mk("o1T8", (P, DC, CUR), F8, "right")
            a1T, fr_a1T = mk("a1T", (P, DC, CUR), F8, "right")
            for np2 in range(DC // 2):
                pp = PB()
                for sub in range(2):
                    n = np2 * 2 + sub
                    sl = pp[:, sub * 512:(sub + 1) * 512]
                    for t in range(4):
                        nc.tensor.matmul(
                            sl, lhsT=wproj[:, 2 * t:2 * t + 2, n * P:(n + 1) * P],
                            rhs=avT[:, 2 * t:2 * t + 2, :],
                            start=(t == 0), stop=(t == 3), perf_mode=DR,
                            skip_group_check=True)
                    # psum = 256*(av@Wproj); a1T = 8*relu(av@Wproj + bproj)
                    ro('s', a1T[:, n, :], sl, 1.0 / WS, bproj8_sb[:, n:n + 1],
                       relu=True)
            fr_avT()

            # LN2 sums accumulate inside GRU1's output loop (s1 = sum o1,
            # s2 = sum o1^2, both [1,512] chains in one psum tile's banks)
            s1t = psum_s.tile([P, 512], F32, name="s1t", tag="ps")
            s2t = psum_s.tile([P, 512], F32, name="s2t", tag="ps")
            s1 = s1t[0:1, :]
            s2 = s2t[0:1, :]
            sqw = ph.enter_context(tc.tile_pool(name="sqw", bufs=2, side="left"))

            def post1(n):
                sq = sqw.tile([P, 512], BF16, name="sq")
                nc.vector.tensor_mul(sq, o1T_b[:, n, :], o1T_b[:, n, :])
                nc.tensor.matmul(s1, lhsT=ones_red, rhs=o1T_b[:, n, :],
                                 start=(n == 0), stop=(n == DC - 1),
                                 skip_group_check=True)
                nc.tensor.matmul(s2, lhsT=ones_red, rhs=sq,
                                 start=(n == 0), stop=(n == DC - 1),
                                 skip_group_check=True)

            _gru(nc, tc, ph, PB, gw_d, 1, a1T, inpT8, inpT_f, nbg1_sb,
                 o1T_f, o1T_b, o1T8, post_n=post1, gwp=gwp1, pre=g1pre)
        fr_a1T(); fr_x1T()

        # ========== Phase 5: LN2 (no affine) -> x2T8 (x8 fp8) ==========
        x2T8, fr_x2T = mk("x2T8", (P, DC, CUR), F8, "right")
        with ExitStack() as ph:
            lw = ph.enter_context(tc.tile_pool(name="lw", bufs=2, side="left"))
            mean = lw.tile([1, 512], F32, name="mean")
            nc.vector.tensor_scalar_mul(mean, s1, 1.0 / D)
            m2m = lw.tile([1, 512], F32, name="m2m")
            nc.vector.tensor_scalar_mul(m2m, s2, 1.0 / D)
            var = lw.tile([1, 512], F32, name="var")
            nc.vector.scalar_tensor_tensor(out=var, in0=mean, scalar=1.0,
                                           in1=mean, op0=AluOp.mult,
                                           op1=AluOp.mult)
            nc.vector.tensor_sub(var, m2m, var)
            # sd8 = sqrt((var+eps)/64) = sd/8 ; recip -> 8/sd
            sd8 = lw.tile([1, 512], F32, name="sd8")
            nc.scalar.activation(out=sd8, in_=var, func=Act.Sqrt,
                                 scale=1.0 / 64.0, bias=eps64_t)
            rstd8 = lw.tile([1, 512], F32, name="rstd8")
            nc.vector.reciprocal(out=rstd8, in_=sd8)
            meanB = lw.tile([P, 512], F32, name="meanB")
            nc.gpsimd.partition_broadcast(meanB, mean)
            rstd8B = lw.tile([P, 512], F32, name="rstd8B")
            nc.gpsimd.partition_broadcast(rstd8B, rstd8)
            meanB2 = bass.AP(tensor=meanB.tensor, offset=meanB.offset,
                             ap=[meanB[:].ap[0], [0, 2], [1, 512]])
            rstd8B2 = bass.AP(tensor=rstd8B.tensor, offset=rstd8B.offset,
                              ap=[rstd8B[:].ap[0], [0, 2], [1, 512]])
            for np2 in range(DC // 2):
                t1 = lw.tile([P, 2, 512], F32, name="t1")
                sl_i = o1T_f[:, 2 * np2:2 * np2 + 2, :]
                sl_o = x2T8[:, 2 * np2:2 * np2 + 2, :]
                if np2 % 2 == 0:
                    nc.vector.tensor_sub(t1, sl_i, meanB2)
                    nc.vector.tensor_mul(sl_o, t1, rstd8B2)
                else:
                    nc.gpsimd.tensor_sub(t1, sl_i, meanB2)
                    nc.gpsimd.tensor_mul(sl_o, t1, rstd8B2)

        # ========== Phase 6: MLP ========================================
        m2T, fr_m2T = mk("m2T", (P, DC, CUR), F8, "left")
        with ExitStack() as ph6:
            w1p = ph6.enter_context(tc.tile_pool(name="w1p", bufs=1, side="right"))
            m1w = ph6.enter_context(tc.tile_pool(name="m1w", bufs=1, side="right"))
            w2p = ph6.enter_context(tc.tile_pool(name="w2p", bufs=1, side="right"))
            w1 = w1p.tile([P, DC, HID], F8)
            w1_ap = w1_d[:].rearrange("(kc p) n -> p kc n", p=P)
            for hf in range(2):
                nc.sync.dma_start(out=w1[:, hf * 4:(hf + 1) * 4, :],
                                  in_=w1_ap[:, hf * 4:(hf + 1) * 4, :])
            w2 = w2p.tile([P, HC, D], F8)
            w2_ap = w2_d[:].rearrange("(kc p) n -> p kc n", p=P)
            for hf in range(2):
                nc.sync.dma_start(out=w2[:, hf * 16:(hf + 1) * 16, :],
                                  in_=w2_ap[:, hf * 16:(hf + 1) * 16, :])
            m1T = m1w.tile([P, HC, 512], F8)
            for np2 in range(HC // 2):
                pp = PB()
                for sub in range(2):
                    n = np2 * 2 + sub
                    sl = pp[:, sub * 512:(sub + 1) * 512]
                    for t in range(4):
                        nc.tensor.matmul(
                            sl, lhsT=w1[:, 2 * t:2 * t + 2, n * P:(n + 1) * P],
                            rhs=x2T8[:, 2 * t:2 * t + 2, :],
                            start=(t == 0), stop=(t == 3), perf_mode=DR,
                            skip_group_check=True)
                    # W1 stored x8, x2 x8 -> psum = 64*(x2@W1f);
                    # m1T = 64*relu(x2@W1f + b1f) via one (add,max) op
                    if n % 2 == 0:
                        nc.vector.tensor_scalar(out=m1T[:, n, :], in0=sl,
                                                scalar1=b1f64_sb[:, n:n + 1],
                                                scalar2=0.0, op0=AluOp.add,
                                                op1=AluOp.max)
                    else:
                        nc.scalar.activation(out=m1T[:, n, :], in_=sl,
                                             func=Act.Relu, scale=1.0,
                                             bias=b1f64_sb[:, n:n + 1])
            for np2 in range(DC // 2):
                pp = PB()
                for sub in range(2):
                    n = np2 * 2 + sub
                    sl = pp[:, sub * 512:(sub + 1) * 512]
                    for t in range(16):
                        nc.tensor.matmul(
                            sl, lhsT=w2[:, 2 * t:2 * t + 2, n * P:(n + 1) * P],
                            rhs=m1T[:, 2 * t:2 * t + 2, :],
                            start=(t == 0), stop=(t == 15), perf_mode=DR,
                            skip_group_check=True)
                    # psum = 64*32*(m1@W2) = 2048x ; m2T = 8*relu(...)
                    ro('s', m2T[:, n, :], sl, 1.0 / 256.0, b2_8_sb[:, n:n + 1],
                       relu=True)
        fr_x2T()

        # ========== Phase 7: GRU2 =======================================
        o2T_f, fr_o2 = mk("o2T_f", (P, DC, CUR), F32, "left")
        with ExitStack() as ph:
            ow = ph.enter_context(tc.tile_pool(name="ow", bufs=2, side="left"))

            def post2(n):
                # transpose this output feature chunk and stream it out
                pt = PS()
                for q in range(TCC):
                    nc.tensor.transpose(pt[:, q * P:(q + 1) * P],
                                        o2T_f[:, n, q * P:(q + 1) * P],
                                        ident_f)
                on = ow.tile([P, TCC, P], F32, name="on")
                if n % 2 == 0:
                    nc.vector.tensor_copy(on, pt[:, :].rearrange(
                        "p (a b) -> p a b", a=TCC))
                else:
                    nc.scalar.copy(on, pt[:, :].rearrange(
                        "p (a b) -> p a b", a=TCC))
                o_ap = bass.AP(tensor=out_d.tensor,
                               offset=out_d.offset + n * P,
                               ap=[[D, P], [P * D, TCC], [1, P]])
                nc.sync.dma_start(out=o_ap, in_=on)

            _gru(nc, tc, ph, PB, gw_d, 2, m2T, o1T8, o1T_f, nbg2_sb,
                 o2T_f, None, None, post_n=post2)
        fr_o18(); fr_o1b(); fr_o1f(); fr_inp8(); fr_inpf()

        fr_o2(); fr_m2T()


def _gru(nc, tc, ph, PB, gw_d, g, yT8, xT8, xT_f, nbg_sb, oT_f, oT_b, oT8,
         post_n=None, gwp=None, pre=None):
    """GRU gate: yT8/xT8 fp8 at 8x scale; psum = 256*(y@W + x@U)."""
    if gwp is None:
        gwp = ph.enter_context(tc.tile_pool(name=f"gw{g}", bufs=3, side="left"))
    gtmp = ph.enter_context(tc.tile_pool(name=f"gt{g}", bufs=2, side="left"))
    gper = ph.enter_context(tc.tile_pool(name=f"gp{g}", bufs=1, side="left"))
    pre = pre or {}

    def loadw(m):
        if m in pre:
            return pre[m]
        w = gwp.tile([P, DC, D], F8, name=f"gwt_{m}", tag="gwt")
        ap = gw_d[(g, m)][:].rearrange("(kc p) n -> p kc n", p=P)
        nc.sync.dma_start(out=w, in_=ap)
        return w

    def gate_psum(ps, w, u, n, rhs_x):
        for t in range(4):
            nc.tensor.matmul(ps, lhsT=w[:, 2 * t:2 * t + 2, n * P:(n + 1) * P],
                             rhs=yT8[:, 2 * t:2 * t + 2, :],
                             start=(t == 0), stop=False, perf_mode=DR,
                             skip_group_check=True)
        for t in range(4):
            nc.tensor.matmul(ps, lhsT=u[:, 2 * t:2 * t + 2, n * P:(n + 1) * P],
                             rhs=rhs_x[:, 2 * t:2 * t + 2, :],
                             start=False, stop=(t == 3), perf_mode=DR,
                             skip_group_check=True)

    wr, ur = loadw("Wr"), loadw("Ur")
    rx8 = gper.tile([P, DC, 512], F8, name="rx8")
    for np2 in range(DC // 2):
        pp = PB()
        for sub in range(2):
            n = np2 * 2 + sub
            sl = pp[:, sub * 512:(sub + 1) * 512]
            gate_psum(sl, wr, ur, n, xT8)
            rr = gtmp.tile([P, 512], F32, name="rr")
            nc.scalar.activation(out=rr, in_=sl, func=Act.Sigmoid, scale=RS)
            # rx8 = (8*r) * x
            nc.vector.scalar_tensor_tensor(out=rx8[:, n, :], in0=rr, scalar=AS,
                                           in1=xT_f[:, n, :], op0=AluOp.mult,
                                           op1=AluOp.mult)
    wz, uz = loadw("Wz"), loadw("Uz")
    zt = gper.tile([P, DC, 512], BF16, name="zt")
    for np2 in range(DC // 2):
        pp = PB()
        for sub in range(2):
            n = np2 * 2 + sub
            sl = pp[:, sub * 512:(sub + 1) * 512]
            gate_psum(sl, wz, uz, n, xT8)
            nc.scalar.activation(out=zt[:, n, :], in_=sl, func=Act.Sigmoid,
                                 scale=RS, bias=nbg_sb[:, n:n + 1])
    wg, ug = loadw("Wg"), loadw("Ug")
    for np2 in range(DC // 2):
        pp = PB()
        for sub in range(2):
            n = np2 * 2 + sub
            sl = pp[:, sub * 512:(sub + 1) * 512]
            gate_psum(sl, wg, ug, n, rx8)
            ht = gtmp.tile([P, 512], BF16, name="ht")
            nc.scalar.activation(out=ht, in_=sl, func=Act.Tanh, scale=RS)
            d1 = gtmp.tile([P, 512], BF16, name="d1")
            nc.gpsimd.tensor_sub(d1, ht, xT_f[:, n, :])
            zd = gtmp.tile([P, 512], BF16, name="zd")
            nc.vector.tensor_mul(zd, d1, zt[:, n, :])
            nc.vector.tensor_add(oT_f[:, n, :], zd, xT_f[:, n, :])
            if oT_b is not None:
                nc.gpsimd.tensor_copy(oT_b[:, n, :], oT_f[:, n, :])
            if oT8 is not None:
                nc.gpsimd.tensor_scalar_mul(oT8[:, n, :], oT_f[:, n, :], AS)
            if post_n is not None:
                post_n(n)


_NC_CACHE = {}


def _get_nc():
    if "nc" not in _NC_CACHE:
        _NC_CACHE["nc"] = _build()
    return _NC_CACHE["nc"]


def _chunk_t(vec):
    n = vec.shape[0] // P
    return np.ascontiguousarray(vec.reshape(n, P).T.astype(np.float32))


def _f8(x, s=WS):
    return np.asarray(np.asarray(x, np.float32) * s, NF8)


def _prep(inputs):
    f32 = np.float32
    inp = np.asarray(inputs["inputs"], f32)
    mem = np.asarray(inputs["memory"], f32)
    pos = np.asarray(inputs["pos_embedding"], f32)[:, 0, :]

    g1 = np.asarray(inputs["ln1_g"], f32)
    b1 = np.asarray(inputs["ln1_b"], f32)
    g2 = np.asarray(inputs["ln2_g"], f32)
    b2 = np.asarray(inputs["ln2_b"], f32)
    Wkv = np.asarray(inputs["Wkv"], f32)
    Wq = np.asarray(inputs["Wq"], f32)
    W1 = np.asarray(inputs["mlp_W1"], f32)

    Wkv_f = g1[:, None] * Wkv
    bkv_f = b1 @ Wkv + np.asarray(inputs["bkv"], f32)
    Wq_f = g1[:, None] * Wq
    bq_f = b1 @ Wq + np.asarray(inputs["bq"], f32)
    W1_f = g2[:, None] * W1
    b1_f = b2 @ W1 + np.asarray(inputs["mlp_b1"], f32)

    u_flat = np.asarray(inputs["u"], f32).reshape(-1)
    v_flat = np.asarray(inputs["v"], f32).reshape(-1)

    shared = {
        "posT": np.ascontiguousarray(pos.T).astype(NF8),
        "Wkv": _f8(Wkv_f), "Wq": _f8(Wq_f), "Wpos": _f8(inputs["Wpos"]),
        "Wproj": _f8(inputs["Wproj"]),
        "mlp_W1": _f8(W1_f, 8.0), "mlp_W2": _f8(inputs["mlp_W2"]),
        "bkvV32_row": (bkv_f[D:2 * D] * WS).reshape(1, D).astype(NBF),
        "biases_t": np.concatenate([
            _chunk_t(bkv_f[0:D]),
            _chunk_t(bq_f + u_flat),
            _chunk_t(bq_f + v_flat),
            _chunk_t(np.asarray(inputs["bpos"], f32)),
            _chunk_t(np.asarray(inputs["bproj"], f32) * AS),
            _chunk_t(np.asarray(inputs["mlp_b2"], f32) * AS),
            _chunk_t(-np.asarray(inputs["g1_bg"], f32)),
            _chunk_t(-np.asarray(inputs["g2_bg"], f32)),
            _chunk_t(b1_f * 64.0),
        ], axis=1),
    }
    for g in (1, 2):
        for m in ("Wr", "Ur", "Wz", "Uz", "Wg", "Ug"):
            shared[f"g{g}_{m}"] = _f8(inputs[f"g{g}_{m}"])

    in_maps = []
    for b in range(BS):
        im = dict(shared)
        im["x_full"] = np.ascontiguousarray(
            np.concatenate([mem[:, b, :], inp[:, b, :]], axis=0)).astype(NBF)
        im["inpT"] = np.ascontiguousarray(inp[:, b, :].T)
        in_maps.append(im)
    return in_maps


def kernel(**inputs):
    nc = _get_nc()
    in_maps = _prep(inputs)
    res = run_bass_kernel_spmd(nc, in_maps, core_ids=list(range(BS)))
    out = np.stack([res.results[b]["out"] for b in range(BS)], axis=1)
    return np.ascontiguousarray(out.astype(np.float32))


if __name__ == "__main__":
    _get_nc()
    print("build+compile OK")
